# revision 1
# baseline (speedup 1.0000x reference)
"""Trainium2 Bass kernel for nn_ConditionalSpline1DFlow (K=16 RQS flow).

Data-parallel over 8 cores (B=4096 -> 512 rows/core). Per core:
  1. Conditioner MLP on TensorE (feature-major).
  2. Spline params per row; rescale bin k's rational-quadratic by
     s_k = delta_0/delta_k so numerator N, denominator D and
     derivative-numerator C become globally CONTINUOUS piecewise
     quadratics in x.
  3. Evaluate N, D, C gather-free in the clipped-ramp basis
        P(x) = const + sum_k a_k*(t_k - x_k)^2 + b_k*(t_k - x_k),
        t_k = clip(x, x_k, x_{k+1})
     on TensorE: rows packed (b*16+k) so one [128, 24] matmul contracts
     all 16 bins x 3 polys for 8 batch rows at once; PSUM accumulates the
     (linear, square) stream pair.
  4. out = N/D + (y - clip(y)); logdet = (ln C - 2 ln D) * (y == clip(y)).
"""
import sys
import numpy as np

K = 16
BOUND = 5.0
MBW = 1e-3
MBH = 1e-3
MD = 1e-3
B_FULL, N = 4096, 1024
CD, H = 64, 256
OUT3 = 3 * K + 1
NCORES = 8
BL = B_FULL // NCORES   # 512 rows per core
T = BL // 128           # 4 partition tiles
G = 128 // 8            # (unused) 8-row groups
GG = 128 // 16          # 8 groups of 16 rows per tile
CH = N // 512           # 2 free-dim chunks

MODE = "t"  # "t": stream clipped-t w/ folded consts; "u": stream t - x_k

_CACHE = {}


def _ensure_path():
    for p in ("/opt/trn_rl_repo",):
        if p not in sys.path:
            sys.path.insert(0, p)


def _build_nc():
    _ensure_path()
    import concourse.bacc as bacc
    import concourse.tile as tile
    from concourse import mybir

    fp32 = mybir.dt.float32
    nc = bacc.Bacc("TRN2", target_bir_lowering=False, debug=False)

    io = dict(
        cond=nc.dram_tensor("cond", [BL, CD], fp32, kind="ExternalInput"),
        y=nc.dram_tensor("y", [BL, N], fp32, kind="ExternalInput"),
        W1=nc.dram_tensor("W1", [CD, H], fp32, kind="ExternalInput"),
        b1=nc.dram_tensor("b1", [H], fp32, kind="ExternalInput"),
        W2=nc.dram_tensor("W2", [H, H], fp32, kind="ExternalInput"),
        b2=nc.dram_tensor("b2", [H], fp32, kind="ExternalInput"),
        W3=nc.dram_tensor("W3", [H, OUT3], fp32, kind="ExternalInput"),
        b3=nc.dram_tensor("b3", [OUT3], fp32, kind="ExternalInput"),
        out=nc.dram_tensor("out", [BL, N], fp32, kind="ExternalOutput"),
        logdet=nc.dram_tensor("logdet", [BL, N], fp32, kind="ExternalOutput"),
    )
    with tile.TileContext(nc) as tc:
        _emit(nc, tc, io)
    nc.compile()
    return nc


def _emit(nc, tc, io):
    from contextlib import ExitStack
    import concourse.bass as bass
    from concourse import mybir

    fp32 = mybir.dt.float32
    i32 = mybir.dt.int32
    AF = mybir.ActivationFunctionType
    OP = mybir.AluOpType
    AX = mybir.AxisListType

    TT = nc.vector.tensor_tensor
    TS = nc.vector.tensor_scalar
    STT = nc.vector.scalar_tensor_tensor
    fp32r = mybir.dt.float32r

    def mmr(out, lhsT, rhs, **kw):
        # fp32r (TF32-like) would be 4x faster on PE but requires rounding
        # every producer to reduced precision; keep exact fp32.
        nc.tensor.matmul(out, lhsT, rhs, **kw)

    ctx = ExitStack()
    with ctx:
        singles = ctx.enter_context(tc.tile_pool(name="singles", bufs=1))
        work = ctx.enter_context(tc.tile_pool(name="work", bufs=3))
        fin = ctx.enter_context(tc.tile_pool(name="fin", bufs=2))
        psum_mm = ctx.enter_context(tc.tile_pool(name="psum_mm", bufs=2, space="PSUM"))
        psum_acc = ctx.enter_context(tc.tile_pool(name="psum_acc", bufs=1, space="PSUM"))
        dscr = ctx.enter_context(tc.tile_pool(name="dscr", bufs=2, space="DRAM"))

        dma = nc.sync.dma_start

        cnt = [0]

        def ps_tile(p, f):
            cnt[0] += 1
            return psum_mm.tile([p, f], fp32, tag="ps", name=f"ps{cnt[0]}")

        # ===== iota-derived constant masks =====
        iota_i = singles.tile([128, 1], i32)
        nc.gpsimd.iota(iota_i, pattern=[[0, 1]], base=0, channel_multiplier=1)
        iota_f = singles.tile([128, 1], fp32)
        nc.vector.tensor_copy(iota_f, iota_i)

        bkf_i = singles.tile([128, 16, 8], i32)   # value b' at col (b'*8+m)
        nc.gpsimd.iota(bkf_i, pattern=[[1, 16], [0, 8]], base=0, channel_multiplier=0)
        bkf_f = singles.tile([128, 16, 8], fp32)
        nc.vector.tensor_copy(bkf_f, bkf_i)

        colf_i = singles.tile([128, 128], i32)    # value j at col j
        nc.gpsimd.iota(colf_i, pattern=[[1, 128]], base=0, channel_multiplier=0)
        colf_f = singles.tile([128, 128], fp32)
        nc.vector.tensor_copy(colf_f, colf_i)

        pmod_i = singles.tile([128, 1], i32)      # p % 16
        TS(pmod_i, iota_i, 15, None, OP.bitwise_and)
        pmod_f = singles.tile([128, 1], fp32)
        nc.vector.tensor_copy(pmod_f, pmod_i)

        ident = singles.tile([128, 128], fp32)    # identity matrix
        TS(ident, colf_f, iota_f, None, OP.is_equal)

        lhsT16 = singles.tile([16, 128], fp32)     # [b, b'*8+m] = (b'==b)
        TS(lhsT16, bkf_f.rearrange("p a b -> p (a b)")[:16], iota_f[:16], None,
           OP.is_equal)

        maskbb = singles.tile([128, 16, 8], fp32)  # [p, (b',m)] = (p%16==b')
        TS(maskbb, bkf_f, pmod_f, None, OP.is_equal)

        # per-group replication masks: repl[gg][p, (b',m)] = (p == 16gg+b')
        repl = singles.tile([128, GG, 16, 8], fp32)
        for g in range(GG):
            pg = work.tile([128, 1], fp32, tag="pg", name="pg")
            TS(pg, iota_f, float(-16 * g), None, OP.add)
            TS(repl[:, g, :, :], bkf_f, pg, None, OP.is_equal)

        ps_h16 = ps_tile(128, 16)
        nc.tensor.transpose(ps_h16, lhsT16, ident[:16, :16])
        H16 = singles.tile([128, 16], fp32)        # [p, b'] = (p//8==b')
        nc.scalar.copy(H16, ps_h16)

        # gsel[p, g] = (p//16 == g); gqsel[p, q] = (p//32 == q)
        pdiv16_i = singles.tile([128, 1], i32)
        TS(pdiv16_i, iota_i, 4, None, OP.arith_shift_right)
        pdiv16_f = singles.tile([128, 1], fp32)
        nc.vector.tensor_copy(pdiv16_f, pdiv16_i)
        col8_i = singles.tile([128, 8], i32)
        nc.gpsimd.iota(col8_i, pattern=[[1, 8]], base=0, channel_multiplier=0)
        col8_f = singles.tile([128, 8], fp32)
        nc.vector.tensor_copy(col8_f, col8_i)
        gsel = singles.tile([128, 8], fp32)
        TS(gsel, col8_f, pdiv16_f, None, OP.is_equal)

        pdiv32_i = singles.tile([128, 1], i32)
        TS(pdiv32_i, iota_i, 5, None, OP.arith_shift_right)
        pdiv32_f = singles.tile([128, 1], fp32)
        nc.vector.tensor_copy(pdiv32_f, pdiv32_i)
        gqsel = singles.tile([128, 4], fp32)
        TS(gqsel, col8_f[:, 0:4], pdiv32_f, None, OP.is_equal)

        # maskC[p, (go',pi',b')] = ((p//16)%2 == go') * (p%16 == b')
        pm2_i = singles.tile([128, 1], i32)
        TS(pm2_i, pdiv16_i, 1, None, OP.bitwise_and)
        pm2_f = singles.tile([128, 1], fp32)
        nc.vector.tensor_copy(pm2_f, pm2_i)
        gof_i = singles.tile([128, 2, 4, 16], i32)
        nc.gpsimd.iota(gof_i, pattern=[[1, 2], [0, 4], [0, 16]], base=0,
                       channel_multiplier=0)
        gof_f = singles.tile([128, 2, 4, 16], fp32)
        nc.vector.tensor_copy(gof_f, gof_i)
        bf2_i = singles.tile([128, 2, 4, 16], i32)
        nc.gpsimd.iota(bf2_i, pattern=[[0, 2], [0, 4], [1, 16]], base=0,
                       channel_multiplier=0)
        bf2_f = singles.tile([128, 2, 4, 16], fp32)
        nc.vector.tensor_copy(bf2_f, bf2_i)
        mgo = singles.tile([128, 2, 4, 16], fp32)
        TS(mgo, gof_f, pm2_f, None, OP.is_equal)
        maskC = singles.tile([128, 2, 4, 16], fp32)
        mb2 = singles.tile([128, 2, 4, 16], fp32)
        TS(mb2, bf2_f, pmod_f, None, OP.is_equal)
        TT(maskC, mgo, mb2, OP.mult)

        # ===== weights =====
        W1s = singles.tile([CD, H], fp32)
        dma(out=W1s, in_=io["W1"][:, :])
        W2s = [singles.tile([128, H], fp32, tag=f"w2_{i}", name=f"w2_{i}") for i in range(2)]
        W3s = [singles.tile([128, OUT3], fp32, tag=f"w3_{i}", name=f"w3_{i}") for i in range(2)]
        for i in range(2):
            dma(out=W2s[i], in_=io["W2"][i * 128:(i + 1) * 128, :])
            dma(out=W3s[i], in_=io["W3"][i * 128:(i + 1) * 128, :])
        b1t = singles.tile([128, 2], fp32)
        dma(out=b1t, in_=io["b1"].rearrange("(h p) -> p h", p=128))
        b2t = singles.tile([128, 2], fp32)
        dma(out=b2t, in_=io["b2"].rearrange("(h p) -> p h", p=128))
        b3t = singles.tile([OUT3, 1], fp32)
        dma(out=b3t, in_=io["b3"].rearrange("(o u) -> o u", u=1))

        # ===== y, xc =====
        y_sb = singles.tile([128, T, N], fp32)
        xc_sb = singles.tile([128, T, N], fp32)
        for t in range(T):
            dma(out=y_sb[:, t, :], in_=io["y"][t * 128:(t + 1) * 128, :])
        for t in range(T):
            nc.gpsimd.tensor_scalar(xc_sb[:, t, :], y_sb[:, t, :], -BOUND, BOUND,
                                    OP.max, OP.min)

        # ===== MLP =====
        condT = singles.tile([CD, BL], fp32)
        for t in range(T):
            csb = work.tile([128, CD], fp32, tag="cond", name="csb")
            dma(out=csb, in_=io["cond"][t * 128:(t + 1) * 128, :])
            ps = ps_tile(CD, 128)
            nc.tensor.transpose(ps, csb, ident)
            nc.scalar.copy(condT[:, t * 128:(t + 1) * 128], ps)

        h1 = [singles.tile([128, BL], fp32, tag=f"h1_{i}", name=f"h1_{i}") for i in range(2)]
        for half in range(2):
            ps = ps_tile(128, BL)
            mmr(ps, W1s[:, half * 128:(half + 1) * 128], condT,
                start=True, stop=True)
            nc.scalar.activation(h1[half], ps, AF.Relu, bias=b1t[:, half:half + 1])
        h2 = [singles.tile([128, BL], fp32, tag=f"h2_{i}", name=f"h2_{i}") for i in range(2)]
        for half in range(2):
            ps = ps_tile(128, BL)
            for kc in range(2):
                mmr(ps, W2s[kc][:, half * 128:(half + 1) * 128], h1[kc],
                    start=(kc == 0), stop=(kc == 1))
            nc.scalar.activation(h2[half], ps, AF.Relu, bias=b2t[:, half:half + 1])
        p_f = singles.tile([OUT3, BL], fp32)
        ps49 = ps_tile(OUT3, BL)
        for kc in range(2):
            mmr(ps49, W3s[kc], h2[kc], start=(kc == 0), stop=(kc == 1))
        nc.scalar.activation(p_f, ps49, AF.Identity, bias=b3t)

        pw = singles.tile([128, T, OUT3], fp32)   # p row-major
        for t in range(T):
            ps = ps_tile(128, OUT3)
            nc.tensor.transpose(ps, p_f[:, t * 128:(t + 1) * 128], ident[:OUT3, :OUT3])
            nc.scalar.copy(pw[:, t, :], ps)

        # ===== param pipeline =====
        un_w = pw[:, :, 0:K]
        un_h = pw[:, :, K:2 * K]
        un_d = pw[:, :, 2 * K:3 * K + 1]

        def softmax_w(un, mb, tag):
            mx = singles.tile([128, T], fp32, tag=f"mx{tag}", name=f"mx{tag}")
            nc.vector.tensor_reduce(mx, un, axis=AX.X, op=OP.max)
            nmx = singles.tile([128, T], fp32, tag=f"nmx{tag}", name=f"nmx{tag}")
            TS(nmx, mx, -1.0, None, OP.mult)
            ein = singles.tile([128, T, K], fp32, tag=f"ein{tag}", name=f"ein{tag}")
            for t in range(T):
                TS(ein[:, t, :], un[:, t, :], nmx[:, t:t + 1], None, OP.add)
            ex = singles.tile([128, T, K], fp32, tag=f"ex{tag}", name=f"ex{tag}")
            nc.scalar.activation(ex, ein, AF.Exp)
            sm = singles.tile([128, T], fp32, tag=f"sm{tag}", name=f"sm{tag}")
            nc.vector.tensor_reduce(sm, ex, axis=AX.X, op=OP.add)
            rs = singles.tile([128, T], fp32, tag=f"rs{tag}", name=f"rs{tag}")
            nc.vector.reciprocal(rs, sm)
            wd = singles.tile([128, T, K], fp32, tag=f"wd{tag}", name=f"wd{tag}")
            for t in range(T):
                TS(wd[:, t, :], ex[:, t, :], rs[:, t:t + 1], 2 * BOUND - K * mb,
                   OP.mult, OP.mult)
            TS(wd, wd, mb, None, OP.add)
            return wd

        widths = softmax_w(un_w, MBW, "w")
        heights = softmax_w(un_h, MBH, "h")

        zeros16 = singles.tile([128, K], fp32)
        nc.vector.memset(zeros16, 0.0)
        cumw = singles.tile([128, T, K + 1], fp32)
        cumh = singles.tile([128, T, K + 1], fp32)
        nc.vector.memset(cumw[:, :, 0:1], -BOUND)
        nc.vector.memset(cumh[:, :, 0:1], -BOUND)
        for t in range(T):
            nc.vector.tensor_tensor_scan(cumw[:, t, 1:], widths[:, t, :], zeros16,
                                         -BOUND, OP.add, OP.add)
            nc.vector.tensor_tensor_scan(cumh[:, t, 1:], heights[:, t, :], zeros16,
                                         -BOUND, OP.add, OP.add)

        # softplus(x) = max(x,0) + ln(1 + exp(-|x|)) (no Softplus table on TRN2)
        deriv = singles.tile([128, T, K + 1], fp32)
        absd = singles.tile([128, T, K + 1], fp32)
        nc.scalar.activation(absd, un_d, AF.Abs)
        end_ = singles.tile([128, T, K + 1], fp32)
        nc.scalar.activation(end_, absd, AF.Exp, scale=-1.0)
        l1p = singles.tile([128, T, K + 1], fp32)
        nc.scalar.activation(l1p, end_, AF.Ln, bias=1.0)
        rl = singles.tile([128, T, K + 1], fp32)
        TS(rl, un_d, 0.0, MD, OP.max, OP.add)
        TT(deriv, rl, l1p, OP.add)

        d0 = deriv[:, :, 0:K]
        d1 = deriv[:, :, 1:K + 1]
        y0 = cumh[:, :, 0:K]
        kx = cumw[:, :, 0:K]
        kx1 = cumw[:, :, 1:K + 1]

        def tmp(tag):
            return singles.tile([128, T, K], fp32, tag=tag, name=tag)

        iw = tmp("iw"); nc.vector.reciprocal(iw, widths)
        delta = tmp("delta"); TT(delta, heights, iw, OP.mult)
        rdelta = tmp("rdelta"); nc.vector.reciprocal(rdelta, delta)
        s = tmp("s")
        for t in range(T):
            TS(s[:, t, :], rdelta[:, t, :], delta[:, t, 0:1], None, OP.mult)
        sig = tmp("sig"); TT(sig, d0, d1, OP.add)
        STT(sig, delta, -2.0, sig, OP.mult, OP.add)
        sdelta = tmp("sdelta"); TT(sdelta, s, delta, OP.mult)
        ssig = tmp("ssig"); TT(ssig, s, sig, OP.mult)
        sh = tmp("sh"); TT(sh, s, heights, OP.mult)
        shd0 = tmp("shd0"); TT(shd0, sh, d0, OP.mult)
        t1 = tmp("t1"); TT(t1, y0, ssig, OP.mult)
        Nc1 = tmp("Nc1"); TT(Nc1, t1, shd0, OP.add)
        u1 = tmp("u1"); TT(u1, delta, d0, OP.subtract)
        u2 = tmp("u2"); TT(u2, sh, u1, OP.mult)
        Nc2 = tmp("Nc2"); TT(Nc2, u2, t1, OP.subtract)
        sd2 = tmp("sd2"); TT(sd2, sdelta, sdelta, OP.mult)
        Cc1 = tmp("Cc1"); STT(Cc1, sd2, 2.0, u1, OP.mult, OP.mult)
        Cc2 = tmp("Cc2"); TT(Cc2, sd2, sig, OP.mult)
        iw2 = tmp("iw2"); TT(iw2, iw, iw, OP.mult)

        # final coefs into one contiguous tile: coefcat[:, t, ci, k]
        # ci: 0=aN 1=bN 2=aD 3=bD 4=aC 5=bC 6=kx 7=kx1
        coefcat = singles.tile([128, T, 8, K], fp32)
        aN = coefcat[:, :, 0, :]; TT(aN, Nc2, iw2, OP.mult)
        bN = coefcat[:, :, 1, :]; TT(bN, Nc1, iw, OP.mult)
        aD = coefcat[:, :, 2, :]; STT(aD, ssig, -1.0, iw2, OP.mult, OP.mult)
        bD = coefcat[:, :, 3, :]; TT(bD, ssig, iw, OP.mult)
        aC = coefcat[:, :, 4, :]; TT(aC, Cc2, iw2, OP.mult)
        bC = coefcat[:, :, 5, :]; TT(bC, Cc1, iw, OP.mult)
        nc.vector.tensor_copy(coefcat[:, :, 6, :], kx)
        nc.vector.tensor_copy(coefcat[:, :, 7, :], kx1)

        # per-row constants, packed 4-wide (pi 3 = 0) for the cpk transform
        constcat4 = singles.tile([128, 4, T], fp32)
        nc.vector.memset(constcat4[:, 3, :], 0.0)
        constN = constcat4[:, 0, :]
        TT(constN, y0[:, :, 0], sdelta[:, :, 0], OP.mult)
        constD = constcat4[:, 1, :]
        nc.vector.tensor_copy(constD, sdelta[:, :, 0])
        constC = constcat4[:, 2, :]
        TT(constC, sd2[:, :, 0], d0[:, :, 0], OP.mult)

        if MODE == "t":
            for cst, b in ((constN, bN), (constD, bD), (constC, bC)):
                bx = tmp("bx"); TT(bx, b, kx, OP.mult)
                sbx = singles.tile([128, T], fp32, tag="sbx", name="sbx")
                nc.vector.tensor_reduce(sbx, bx, axis=AX.X, op=OP.add)
                TT(cst, cst, sbx, OP.subtract)

        # ===== repack coefficients to (b*8+m) partition layout, k = 8h+m ====
        # PACKN[p=(b*8+m), t, ci, h, g] = coefcat[16g+b, t, ci, 8h+m]
        # via PE: PACK = (coef-expand * maskbb)^T @ gsel  (contraction over
        # the 128 source rows; gsel selects the group).
        PACKN = singles.tile([128, T, 8, 2, GG], fp32)
        for t in range(T):
            psp = ps_tile(128, 128)
            for h in range(2):
                exbig = work.tile([128, 8, 16, 8], fp32, tag="exbig",
                                  name="exbig")
                in0 = coefcat[:, t, :, 8 * h:8 * h + 8].unsqueeze(2)\
                    .broadcast_to([128, 8, 16, 8])
                in1 = maskbb.unsqueeze(1).broadcast_to([128, 8, 16, 8])
                TT(exbig, in0, in1, OP.mult)
                for ci in range(8):
                    lhs = exbig[:, ci, :, :].rearrange("p a b -> p (a b)")
                    nc.tensor.matmul(psp[:, (ci * 2 + h) * 8:(ci * 2 + h) * 8 + 8],
                                     lhs, gsel, start=True, stop=True)
            nc.scalar.copy(
                PACKN[:, t, :, :, :].rearrange("p a b c -> p (a b c)"), psp)
        NEGKX = singles.tile([128, T, 2, GG], fp32)
        TS(NEGKX, PACKN[:, :, 6, :, :], -1.0, None, OP.mult)

        # cpk[p=(go*64+pi*16+b), t, gq] = const_pi[32gq+16go+b, t] (SACC layout)
        CPK = singles.tile([128, T, 4], fp32)
        psc = ps_tile(128, 16)
        for t in range(T):
            cE = work.tile([128, 2, 4, 16], fp32, tag="cE", name="cE")
            cin = constcat4[:, :, t].unsqueeze(1).unsqueeze(3)\
                .broadcast_to([128, 2, 4, 16])
            TT(cE, cin, maskC, OP.mult)
            nc.tensor.matmul(psc[:, t * 4:(t + 1) * 4],
                             cE.rearrange("p a b c -> p (a b c)"), gqsel,
                             start=True, stop=True)
        nc.scalar.copy(CPK.rearrange("p a b -> p (a b)"), psc)

        # lhsT mega: [128, T, 2, GG, 4, 16]; per (t,h,g) a contiguous
        # [4poly, 16b'] = 64-col block (poly 3 = zeros, pads po to 64 so
        # start=True initializes the full PSUM slot)
        LHS_L = singles.tile([128, T, 2, GG, 4, 16], fp32)
        LHS_Q = singles.tile([128, T, 2, GG, 4, 16], fp32)
        nc.vector.memset(LHS_L[:, :, :, :, 3, :], 0.0)
        nc.vector.memset(LHS_Q[:, :, :, :, 3, :], 0.0)
        for t in range(T):
            for h in range(2):
                for pi, (lin_c, sq_c) in enumerate(((1, 0), (3, 2), (5, 4))):
                    for dst, ci in ((LHS_L, lin_c), (LHS_Q, sq_c)):
                        csrc = PACKN[:, t, ci, h, :]  # [128, GG]
                        bcs = csrc.unsqueeze(2).broadcast_to([128, GG, 16])
                        h16b = H16.unsqueeze(1).broadcast_to([128, GG, 16])
                        TT(dst[:, t, h, :, pi, :], bcs, h16b, OP.mult)

        # ===== main loop =====
        for t in range(T):
            for c in range(CH):
                ACC = psum_acc.tile([128, 4 * 512], fp32, name="ACC")
                accv = ACC.rearrange("(go pb) (gq n) -> go pb gq n", pb=64, n=512)
                for g in range(GG):
                    xrep = psum_mm.tile([128, 512], fp32, tag="xrep", name="xrep")
                    mmr(xrep, repl[:, g, :, :].rearrange("p a b -> p (a b)"),
                        xc_sb[:, t, c * 512:(c + 1) * 512],
                        start=True, stop=True)
                    slot = ACC[(g % 2) * 64:(g % 2) * 64 + 64,
                               (g // 2) * 512:(g // 2) * 512 + 512]
                    for h in range(2):
                        tk = work.tile([128, 512], fp32, tag="tk", name="tk")
                        TS(tk, xrep, PACKN[:, t, 6, h, g:g + 1],
                           PACKN[:, t, 7, h, g:g + 1], OP.max, OP.min)
                        usq = work.tile([128, 512], fp32, tag="usq", name="usq")
                        nc.scalar.activation(usq, tk, AF.Square,
                                             bias=NEGKX[:, t, h, g:g + 1])
                        if MODE == "u":
                            ulin = work.tile([128, 512], fp32, tag="ulin",
                                             name="ulin")
                            TS(ulin, tk, NEGKX[:, t, h, g:g + 1], None, OP.add)
                            lin_rhs = ulin
                        else:
                            lin_rhs = tk
                        ll = LHS_L[:, t, h, g, :, :].rearrange("p a b -> p (a b)")
                        lq = LHS_Q[:, t, h, g, :, :].rearrange("p a b -> p (a b)")
                        mmr(slot, ll, lin_rhs, start=(h == 0), stop=False)
                        mmr(slot, lq, usq, start=False, stop=(h == 1))

                # PSUM -> SBUF with per-row consts folded in (DMA can't
                # read PSUM); copies split across ACT/DVE
                SACC = fin.tile([128, 4, 512], fp32, tag="SACC", name="SACC")
                for bank in range(4):
                    if bank != 1:
                        nc.scalar.activation(SACC[:, bank, :],
                                             ACC[:, bank * 512:(bank + 1) * 512],
                                             AF.Identity,
                                             bias=CPK[:, t, bank:bank + 1])
                    else:
                        TS(SACC[:, bank, :], ACC[:, bank * 512:(bank + 1) * 512],
                           CPK[:, t, bank:bank + 1], None, OP.add)
                # bounce through DRAM to un-interleave (poly, b) rows:
                # 6 scattered writes + 3 contiguous reads beat 24 direct DMAs
                D1 = dscr.tile([3, 128, 512], fp32, name="D1")
                for go in range(2):
                    for pi in range(3):
                        psrc = SACC[go * 64 + pi * 16:go * 64 + pi * 16 + 16, :, :]
                        dview = bass.AP(
                            tensor=D1.tensor,
                            offset=D1.offset + pi * 128 * 512 + go * 16 * 512,
                            ap=[[512, 16], [32 * 512, 4], [1, 512]])
                        dmax = dma if (go * 3 + pi) % 2 == 0 else nc.scalar.dma_start
                        dmax(out=dview, in_=psrc)
                polys = []
                for pi in range(3):
                    dstt = fin.tile([128, 512], fp32, tag=f"poly{pi}",
                                    name=f"poly{pi}")
                    dmax = dma if pi % 2 == 0 else nc.scalar.dma_start
                    dmax(out=dstt, in_=D1[pi, :, :])
                    polys.append(dstt)
                Np, Dp, Cp = polys

                # finale (ee/inz/outF/ldF on the otherwise-idle GPSIMD)
                ysl = y_sb[:, t, c * 512:(c + 1) * 512]
                xsl = xc_sb[:, t, c * 512:(c + 1) * 512]
                Cm = fin.tile([128, 512], fp32, tag="Cm", name="Cm")
                nc.gpsimd.tensor_scalar(Cm, Cp, 1e-12, None, OP.max)
                rD = fin.tile([128, 512], fp32, tag="rD", name="rD")
                nc.vector.reciprocal(rD, Dp)
                out0 = fin.tile([128, 512], fp32, tag="out0", name="out0")
                nc.gpsimd.tensor_tensor(out0, Np, rD, OP.mult)
                logD = fin.tile([128, 512], fp32, tag="logD", name="logD")
                nc.scalar.activation(logD, Dp, AF.Ln)
                logC = fin.tile([128, 512], fp32, tag="logC", name="logC")
                nc.scalar.activation(logC, Cm, AF.Ln)
                ld0 = fin.tile([128, 512], fp32, tag="ld0", name="ld0")
                STT(ld0, logD, -2.0, logC, OP.mult, OP.add)
                ee = fin.tile([128, 512], fp32, tag="ee", name="ee")
                nc.gpsimd.tensor_tensor(ee, ysl, xsl, OP.subtract)
                inz = fin.tile([128, 512], fp32, tag="inz", name="inz")
                nc.gpsimd.tensor_scalar(inz, ee, 0.0, None, OP.is_equal)
                outF = fin.tile([128, 512], fp32, tag="outF", name="outF")
                nc.gpsimd.tensor_tensor(outF, out0, ee, OP.add)
                ldF = fin.tile([128, 512], fp32, tag="ldF", name="ldF")
                nc.gpsimd.tensor_tensor(ldF, ld0, inz, OP.mult)
                dma(out=io["out"][t * 128:(t + 1) * 128, c * 512:(c + 1) * 512],
                    in_=outF)
                nc.scalar.dma_start(
                    out=io["logdet"][t * 128:(t + 1) * 128, c * 512:(c + 1) * 512],
                    in_=ldF)


def kernel(cond, y, W1, b1, W2, b2, W3, b3):
    _ensure_path()
    from concourse.bass_utils import run_bass_kernel_spmd

    if "nc" not in _CACHE:
        _CACHE["nc"] = _build_nc()
    nc = _CACHE["nc"]

    cond = np.ascontiguousarray(cond, np.float32)
    y = np.ascontiguousarray(y, np.float32)
    shared = dict(W1=np.ascontiguousarray(W1, np.float32),
                  b1=np.ascontiguousarray(b1, np.float32),
                  W2=np.ascontiguousarray(W2, np.float32),
                  b2=np.ascontiguousarray(b2, np.float32),
                  W3=np.ascontiguousarray(W3, np.float32),
                  b3=np.ascontiguousarray(b3, np.float32))
    in_maps = []
    for i in range(NCORES):
        sl = slice(i * BL, (i + 1) * BL)
        in_maps.append(dict(cond=cond[sl], y=y[sl], **shared))
    res = run_bass_kernel_spmd(nc, in_maps, core_ids=list(range(NCORES)))
    out = np.concatenate([r["out"] for r in res.results], axis=0)
    ld = np.concatenate([r["logdet"] for r in res.results], axis=0)
    return out, ld



# revision 2
# speedup vs baseline: 1.5273x; 1.5273x over previous
"""Trainium2 Bass kernel for nn_ConditionalSpline1DFlow (K=16 RQS flow).

Data-parallel over 8 cores (B=4096 -> 512 rows/core). Per core:
  1. Conditioner MLP on TensorE (feature-major).
  2. Spline params per row; rescale bin k's rational-quadratic by
     s_k = delta_0/delta_k so numerator N, denominator D and
     derivative-numerator C become globally CONTINUOUS piecewise
     quadratics in x.
  3. Evaluate N, D, C gather-free in the clipped-ramp basis
        P(x) = const + sum_k a_k*(t_k - x_k)^2 + b_k*(t_k - x_k),
        t_k = clip(x, x_k, x_{k+1})
     on TensorE: rows packed (b*16+k) so one [128, 24] matmul contracts
     all 16 bins x 3 polys for 8 batch rows at once; PSUM accumulates the
     (linear, square) stream pair.
  4. out = N/D + (y - clip(y)); logdet = (ln C - 2 ln D) * (y == clip(y)).
"""
import sys
import numpy as np

K = 16
BOUND = 5.0
MBW = 1e-3
MBH = 1e-3
MD = 1e-3
B_FULL, N = 4096, 1024
CD, H = 64, 256
OUT3 = 3 * K + 1
NCORES = 8
BL = B_FULL // NCORES   # 512 rows per core
T = BL // 128           # 4 partition tiles
G = 128 // 8            # (unused) 8-row groups
GG = 128 // 16          # 8 groups of 16 rows per tile
CH = N // 512           # 2 free-dim chunks

MODE = "t"  # "t": stream clipped-t w/ folded consts; "u": stream t - x_k

_CACHE = {}


def _ensure_path():
    for p in ("/opt/trn_rl_repo",):
        if p not in sys.path:
            sys.path.insert(0, p)


def _build_nc():
    _ensure_path()
    import concourse.bacc as bacc
    import concourse.tile as tile
    from concourse import mybir

    fp32 = mybir.dt.float32
    nc = bacc.Bacc("TRN2", target_bir_lowering=False, debug=False)

    io = dict(
        cond=nc.dram_tensor("cond", [BL, CD], fp32, kind="ExternalInput"),
        y=nc.dram_tensor("y", [BL, N], fp32, kind="ExternalInput"),
        W1=nc.dram_tensor("W1", [CD, H], fp32, kind="ExternalInput"),
        b1=nc.dram_tensor("b1", [H], fp32, kind="ExternalInput"),
        W2=nc.dram_tensor("W2", [H, H], fp32, kind="ExternalInput"),
        b2=nc.dram_tensor("b2", [H], fp32, kind="ExternalInput"),
        W3=nc.dram_tensor("W3", [H, OUT3], fp32, kind="ExternalInput"),
        b3=nc.dram_tensor("b3", [OUT3], fp32, kind="ExternalInput"),
        out=nc.dram_tensor("out", [BL, N], fp32, kind="ExternalOutput"),
        logdet=nc.dram_tensor("logdet", [BL, N], fp32, kind="ExternalOutput"),
    )
    with tile.TileContext(nc) as tc:
        _emit(nc, tc, io)
    nc.compile()
    return nc


def _emit(nc, tc, io):
    from contextlib import ExitStack
    import concourse.bass as bass
    from concourse import mybir

    fp32 = mybir.dt.float32
    i32 = mybir.dt.int32
    AF = mybir.ActivationFunctionType
    OP = mybir.AluOpType
    AX = mybir.AxisListType

    TT = nc.vector.tensor_tensor
    TS = nc.vector.tensor_scalar
    STT = nc.vector.scalar_tensor_tensor
    fp32r = mybir.dt.float32r

    def mmr(out, lhsT, rhs, **kw):
        # fp32r (TF32-like): 4x faster on PE when the moving free dim >= 256.
        # Rel-err budget is 2e-2; TF32 rounding costs ~1e-3 — well inside.
        nc.tensor.matmul(out, lhsT.bitcast(fp32r), rhs.bitcast(fp32r), **kw)

    ctx = ExitStack()
    with ctx:
        singles = ctx.enter_context(tc.tile_pool(name="singles", bufs=1))
        work = ctx.enter_context(tc.tile_pool(name="work", bufs=3))
        fin = ctx.enter_context(tc.tile_pool(name="fin", bufs=2))
        psum_mm = ctx.enter_context(tc.tile_pool(name="psum_mm", bufs=2, space="PSUM"))
        psum_acc = ctx.enter_context(tc.tile_pool(name="psum_acc", bufs=1, space="PSUM"))
        dscr = ctx.enter_context(tc.tile_pool(name="dscr", bufs=2, space="DRAM"))

        dma = nc.sync.dma_start

        cnt = [0]

        def ps_tile(p, f):
            cnt[0] += 1
            return psum_mm.tile([p, f], fp32, tag="ps", name=f"ps{cnt[0]}")

        # ===== iota-derived constant masks =====
        iota_i = singles.tile([128, 1], i32)
        nc.gpsimd.iota(iota_i, pattern=[[0, 1]], base=0, channel_multiplier=1)
        iota_f = singles.tile([128, 1], fp32)
        nc.vector.tensor_copy(iota_f, iota_i)

        bkf_i = singles.tile([128, 16, 8], i32)   # value b' at col (b'*8+m)
        nc.gpsimd.iota(bkf_i, pattern=[[1, 16], [0, 8]], base=0, channel_multiplier=0)
        bkf_f = singles.tile([128, 16, 8], fp32)
        nc.vector.tensor_copy(bkf_f, bkf_i)

        colf_i = singles.tile([128, 128], i32)    # value j at col j
        nc.gpsimd.iota(colf_i, pattern=[[1, 128]], base=0, channel_multiplier=0)
        colf_f = singles.tile([128, 128], fp32)
        nc.vector.tensor_copy(colf_f, colf_i)

        pmod_i = singles.tile([128, 1], i32)      # p % 16
        TS(pmod_i, iota_i, 15, None, OP.bitwise_and)
        pmod_f = singles.tile([128, 1], fp32)
        nc.vector.tensor_copy(pmod_f, pmod_i)

        ident = singles.tile([128, 128], fp32)    # identity matrix
        TS(ident, colf_f, iota_f, None, OP.is_equal)

        lhsT16 = singles.tile([16, 128], fp32)     # [b, b'*8+m] = (b'==b)
        TS(lhsT16, bkf_f.rearrange("p a b -> p (a b)")[:16], iota_f[:16], None,
           OP.is_equal)

        maskbb = singles.tile([128, 16, 8], fp32)  # [p, (b',m)] = (p%16==b')
        TS(maskbb, bkf_f, pmod_f, None, OP.is_equal)

        # per-group replication masks: repl[gg][p, (b',m)] = (p == 16gg+b')
        repl = singles.tile([128, GG, 16, 8], fp32)
        for g in range(GG):
            pg = work.tile([128, 1], fp32, tag="pg", name="pg")
            TS(pg, iota_f, float(-16 * g), None, OP.add)
            TS(repl[:, g, :, :], bkf_f, pg, None, OP.is_equal)

        ps_h16 = ps_tile(128, 16)
        nc.tensor.transpose(ps_h16, lhsT16, ident[:16, :16])
        H16 = singles.tile([128, 16], fp32)        # [p, b'] = (p//8==b')
        nc.scalar.copy(H16, ps_h16)

        # gsel[p, g] = (p//16 == g); gqsel[p, q] = (p//32 == q)
        pdiv16_i = singles.tile([128, 1], i32)
        TS(pdiv16_i, iota_i, 4, None, OP.arith_shift_right)
        pdiv16_f = singles.tile([128, 1], fp32)
        nc.vector.tensor_copy(pdiv16_f, pdiv16_i)
        col8_i = singles.tile([128, 8], i32)
        nc.gpsimd.iota(col8_i, pattern=[[1, 8]], base=0, channel_multiplier=0)
        col8_f = singles.tile([128, 8], fp32)
        nc.vector.tensor_copy(col8_f, col8_i)
        gsel = singles.tile([128, 8], fp32)
        TS(gsel, col8_f, pdiv16_f, None, OP.is_equal)

        pdiv32_i = singles.tile([128, 1], i32)
        TS(pdiv32_i, iota_i, 5, None, OP.arith_shift_right)
        pdiv32_f = singles.tile([128, 1], fp32)
        nc.vector.tensor_copy(pdiv32_f, pdiv32_i)
        gqsel = singles.tile([128, 4], fp32)
        TS(gqsel, col8_f[:, 0:4], pdiv32_f, None, OP.is_equal)

        # maskC[p, (go',pi',b')] = ((p//16)%2 == go') * (p%16 == b')
        pm2_i = singles.tile([128, 1], i32)
        TS(pm2_i, pdiv16_i, 1, None, OP.bitwise_and)
        pm2_f = singles.tile([128, 1], fp32)
        nc.vector.tensor_copy(pm2_f, pm2_i)
        gof_i = singles.tile([128, 2, 4, 16], i32)
        nc.gpsimd.iota(gof_i, pattern=[[1, 2], [0, 4], [0, 16]], base=0,
                       channel_multiplier=0)
        gof_f = singles.tile([128, 2, 4, 16], fp32)
        nc.vector.tensor_copy(gof_f, gof_i)
        bf2_i = singles.tile([128, 2, 4, 16], i32)
        nc.gpsimd.iota(bf2_i, pattern=[[0, 2], [0, 4], [1, 16]], base=0,
                       channel_multiplier=0)
        bf2_f = singles.tile([128, 2, 4, 16], fp32)
        nc.vector.tensor_copy(bf2_f, bf2_i)
        mgo = singles.tile([128, 2, 4, 16], fp32)
        TS(mgo, gof_f, pm2_f, None, OP.is_equal)
        maskC = singles.tile([128, 2, 4, 16], fp32)
        mb2 = singles.tile([128, 2, 4, 16], fp32)
        TS(mb2, bf2_f, pmod_f, None, OP.is_equal)
        TT(maskC, mgo, mb2, OP.mult)

        # ===== weights =====
        W1s = singles.tile([CD, H], fp32)
        dma(out=W1s, in_=io["W1"][:, :])
        W2s = [singles.tile([128, H], fp32, tag=f"w2_{i}", name=f"w2_{i}") for i in range(2)]
        W3s = [singles.tile([128, OUT3], fp32, tag=f"w3_{i}", name=f"w3_{i}") for i in range(2)]
        for i in range(2):
            dma(out=W2s[i], in_=io["W2"][i * 128:(i + 1) * 128, :])
            dma(out=W3s[i], in_=io["W3"][i * 128:(i + 1) * 128, :])
        b1t = singles.tile([128, 2], fp32)
        dma(out=b1t, in_=io["b1"].rearrange("(h p) -> p h", p=128))
        b2t = singles.tile([128, 2], fp32)
        dma(out=b2t, in_=io["b2"].rearrange("(h p) -> p h", p=128))
        b3t = singles.tile([OUT3, 1], fp32)
        dma(out=b3t, in_=io["b3"].rearrange("(o u) -> o u", u=1))

        # ===== y, xc =====
        y_sb = singles.tile([128, T, N], fp32)
        xc_sb = singles.tile([128, T, N], fp32)
        for t in range(T):
            dma(out=y_sb[:, t, :], in_=io["y"][t * 128:(t + 1) * 128, :])
        for t in range(T):
            nc.gpsimd.tensor_scalar(xc_sb[:, t, :], y_sb[:, t, :], -BOUND, BOUND,
                                    OP.max, OP.min)

        # ===== MLP =====
        condT = singles.tile([CD, BL], fp32)
        for t in range(T):
            csb = work.tile([128, CD], fp32, tag="cond", name="csb")
            dma(out=csb, in_=io["cond"][t * 128:(t + 1) * 128, :])
            ps = ps_tile(CD, 128)
            nc.tensor.transpose(ps, csb, ident)
            nc.scalar.copy(condT[:, t * 128:(t + 1) * 128], ps)

        h1 = [singles.tile([128, BL], fp32, tag=f"h1_{i}", name=f"h1_{i}") for i in range(2)]
        for half in range(2):
            ps = ps_tile(128, BL)
            mmr(ps, W1s[:, half * 128:(half + 1) * 128], condT,
                start=True, stop=True)
            nc.scalar.activation(h1[half], ps, AF.Relu, bias=b1t[:, half:half + 1])
        h2 = [singles.tile([128, BL], fp32, tag=f"h2_{i}", name=f"h2_{i}") for i in range(2)]
        for half in range(2):
            ps = ps_tile(128, BL)
            for kc in range(2):
                mmr(ps, W2s[kc][:, half * 128:(half + 1) * 128], h1[kc],
                    start=(kc == 0), stop=(kc == 1))
            nc.scalar.activation(h2[half], ps, AF.Relu, bias=b2t[:, half:half + 1])
        p_f = singles.tile([OUT3, BL], fp32)
        ps49 = ps_tile(OUT3, BL)
        for kc in range(2):
            mmr(ps49, W3s[kc], h2[kc], start=(kc == 0), stop=(kc == 1))
        nc.scalar.activation(p_f, ps49, AF.Identity, bias=b3t)

        pw = singles.tile([128, T, OUT3], fp32)   # p row-major
        for t in range(T):
            ps = ps_tile(128, OUT3)
            nc.tensor.transpose(ps, p_f[:, t * 128:(t + 1) * 128], ident[:OUT3, :OUT3])
            nc.scalar.copy(pw[:, t, :], ps)

        # ===== param pipeline =====
        un_w = pw[:, :, 0:K]
        un_h = pw[:, :, K:2 * K]
        un_d = pw[:, :, 2 * K:3 * K + 1]

        def softmax_w(un, mb, tag):
            mx = singles.tile([128, T], fp32, tag=f"mx{tag}", name=f"mx{tag}")
            nc.vector.tensor_reduce(mx, un, axis=AX.X, op=OP.max)
            nmx = singles.tile([128, T], fp32, tag=f"nmx{tag}", name=f"nmx{tag}")
            TS(nmx, mx, -1.0, None, OP.mult)
            ein = singles.tile([128, T, K], fp32, tag=f"ein{tag}", name=f"ein{tag}")
            for t in range(T):
                TS(ein[:, t, :], un[:, t, :], nmx[:, t:t + 1], None, OP.add)
            ex = singles.tile([128, T, K], fp32, tag=f"ex{tag}", name=f"ex{tag}")
            nc.scalar.activation(ex, ein, AF.Exp)
            sm = singles.tile([128, T], fp32, tag=f"sm{tag}", name=f"sm{tag}")
            nc.vector.tensor_reduce(sm, ex, axis=AX.X, op=OP.add)
            rs = singles.tile([128, T], fp32, tag=f"rs{tag}", name=f"rs{tag}")
            nc.vector.reciprocal(rs, sm)
            wd = singles.tile([128, T, K], fp32, tag=f"wd{tag}", name=f"wd{tag}")
            for t in range(T):
                TS(wd[:, t, :], ex[:, t, :], rs[:, t:t + 1], 2 * BOUND - K * mb,
                   OP.mult, OP.mult)
            TS(wd, wd, mb, None, OP.add)
            return wd

        widths = softmax_w(un_w, MBW, "w")
        heights = softmax_w(un_h, MBH, "h")

        zeros16 = singles.tile([128, K], fp32)
        nc.vector.memset(zeros16, 0.0)
        cumw = singles.tile([128, T, K + 1], fp32)
        cumh = singles.tile([128, T, K + 1], fp32)
        nc.vector.memset(cumw[:, :, 0:1], -BOUND)
        nc.vector.memset(cumh[:, :, 0:1], -BOUND)
        for t in range(T):
            nc.vector.tensor_tensor_scan(cumw[:, t, 1:], widths[:, t, :], zeros16,
                                         -BOUND, OP.add, OP.add)
            nc.vector.tensor_tensor_scan(cumh[:, t, 1:], heights[:, t, :], zeros16,
                                         -BOUND, OP.add, OP.add)

        # softplus(x) = max(x,0) + ln(1 + exp(-|x|)) (no Softplus table on TRN2)
        deriv = singles.tile([128, T, K + 1], fp32)
        absd = singles.tile([128, T, K + 1], fp32)
        nc.scalar.activation(absd, un_d, AF.Abs)
        end_ = singles.tile([128, T, K + 1], fp32)
        nc.scalar.activation(end_, absd, AF.Exp, scale=-1.0)
        l1p = singles.tile([128, T, K + 1], fp32)
        nc.scalar.activation(l1p, end_, AF.Ln, bias=1.0)
        rl = singles.tile([128, T, K + 1], fp32)
        TS(rl, un_d, 0.0, MD, OP.max, OP.add)
        TT(deriv, rl, l1p, OP.add)

        d0 = deriv[:, :, 0:K]
        d1 = deriv[:, :, 1:K + 1]
        y0 = cumh[:, :, 0:K]
        kx = cumw[:, :, 0:K]
        kx1 = cumw[:, :, 1:K + 1]

        def tmp(tag):
            return singles.tile([128, T, K], fp32, tag=tag, name=tag)

        iw = tmp("iw"); nc.vector.reciprocal(iw, widths)
        delta = tmp("delta"); TT(delta, heights, iw, OP.mult)
        rdelta = tmp("rdelta"); nc.vector.reciprocal(rdelta, delta)
        s = tmp("s")
        for t in range(T):
            TS(s[:, t, :], rdelta[:, t, :], delta[:, t, 0:1], None, OP.mult)
        sig = tmp("sig"); TT(sig, d0, d1, OP.add)
        STT(sig, delta, -2.0, sig, OP.mult, OP.add)
        sdelta = tmp("sdelta"); TT(sdelta, s, delta, OP.mult)
        ssig = tmp("ssig"); TT(ssig, s, sig, OP.mult)
        sh = tmp("sh"); TT(sh, s, heights, OP.mult)
        shd0 = tmp("shd0"); TT(shd0, sh, d0, OP.mult)
        t1 = tmp("t1"); TT(t1, y0, ssig, OP.mult)
        Nc1 = tmp("Nc1"); TT(Nc1, t1, shd0, OP.add)
        u1 = tmp("u1"); TT(u1, delta, d0, OP.subtract)
        u2 = tmp("u2"); TT(u2, sh, u1, OP.mult)
        Nc2 = tmp("Nc2"); TT(Nc2, u2, t1, OP.subtract)
        sd2 = tmp("sd2"); TT(sd2, sdelta, sdelta, OP.mult)
        Cc1 = tmp("Cc1"); STT(Cc1, sd2, 2.0, u1, OP.mult, OP.mult)
        Cc2 = tmp("Cc2"); TT(Cc2, sd2, sig, OP.mult)
        iw2 = tmp("iw2"); TT(iw2, iw, iw, OP.mult)

        # final coefs into one contiguous tile: coefcat[:, t, ci, k]
        # ci: 0=aN 1=bN 2=aD 3=bD 4=aC 5=bC 6=kx 7=kx1
        coefcat = singles.tile([128, T, 8, K], fp32)
        aN = coefcat[:, :, 0, :]; TT(aN, Nc2, iw2, OP.mult)
        bN = coefcat[:, :, 1, :]; TT(bN, Nc1, iw, OP.mult)
        aD = coefcat[:, :, 2, :]; STT(aD, ssig, -1.0, iw2, OP.mult, OP.mult)
        bD = coefcat[:, :, 3, :]; TT(bD, ssig, iw, OP.mult)
        aC = coefcat[:, :, 4, :]; TT(aC, Cc2, iw2, OP.mult)
        bC = coefcat[:, :, 5, :]; TT(bC, Cc1, iw, OP.mult)
        nc.vector.tensor_copy(coefcat[:, :, 6, :], kx)
        nc.vector.tensor_copy(coefcat[:, :, 7, :], kx1)

        # per-row constants, packed 4-wide (pi 3 = 0) for the cpk transform
        constcat4 = singles.tile([128, 4, T], fp32)
        nc.vector.memset(constcat4[:, 3, :], 0.0)
        constN = constcat4[:, 0, :]
        TT(constN, y0[:, :, 0], sdelta[:, :, 0], OP.mult)
        constD = constcat4[:, 1, :]
        nc.vector.tensor_copy(constD, sdelta[:, :, 0])
        constC = constcat4[:, 2, :]
        TT(constC, sd2[:, :, 0], d0[:, :, 0], OP.mult)

        if MODE == "t":
            for cst, b in ((constN, bN), (constD, bD), (constC, bC)):
                bx = tmp("bx"); TT(bx, b, kx, OP.mult)
                sbx = singles.tile([128, T], fp32, tag="sbx", name="sbx")
                nc.vector.tensor_reduce(sbx, bx, axis=AX.X, op=OP.add)
                TT(cst, cst, sbx, OP.subtract)

        # ===== repack coefficients to (b*8+m) partition layout, k = 8h+m ====
        # PACKN[p=(b*8+m), t, ci, h, g] = coefcat[16g+b, t, ci, 8h+m]
        # via PE: PACK = (coef-expand * maskbb)^T @ gsel  (contraction over
        # the 128 source rows; gsel selects the group).
        PACKN = singles.tile([128, T, 8, 2, GG], fp32)
        for t in range(T):
            psp = ps_tile(128, 128)
            for h in range(2):
                exbig = work.tile([128, 8, 16, 8], fp32, tag="exbig",
                                  name="exbig")
                in0 = coefcat[:, t, :, 8 * h:8 * h + 8].unsqueeze(2)\
                    .broadcast_to([128, 8, 16, 8])
                in1 = maskbb.unsqueeze(1).broadcast_to([128, 8, 16, 8])
                TT(exbig, in0, in1, OP.mult)
                for ci in range(8):
                    lhs = exbig[:, ci, :, :].rearrange("p a b -> p (a b)")
                    nc.tensor.matmul(psp[:, (ci * 2 + h) * 8:(ci * 2 + h) * 8 + 8],
                                     lhs, gsel, start=True, stop=True)
            nc.scalar.copy(
                PACKN[:, t, :, :, :].rearrange("p a b c -> p (a b c)"), psp)
        NEGKX = singles.tile([128, T, 2, GG], fp32)
        TS(NEGKX, PACKN[:, :, 6, :, :], -1.0, None, OP.mult)

        # cpk[p=(go*64+pi*16+b), t, gq] = const_pi[32gq+16go+b, t] (SACC layout)
        CPK = singles.tile([128, T, 4], fp32)
        psc = ps_tile(128, 16)
        for t in range(T):
            cE = work.tile([128, 2, 4, 16], fp32, tag="cE", name="cE")
            cin = constcat4[:, :, t].unsqueeze(1).unsqueeze(3)\
                .broadcast_to([128, 2, 4, 16])
            TT(cE, cin, maskC, OP.mult)
            nc.tensor.matmul(psc[:, t * 4:(t + 1) * 4],
                             cE.rearrange("p a b c -> p (a b c)"), gqsel,
                             start=True, stop=True)
        nc.scalar.copy(CPK.rearrange("p a b -> p (a b)"), psc)

        # lhsT mega: [128, T, 2, GG, 4, 16]; per (t,h,g) a contiguous
        # [4poly, 16b'] = 64-col block (poly 3 = zeros, pads po to 64 so
        # start=True initializes the full PSUM slot)
        LHS_L = singles.tile([128, T, 2, GG, 4, 16], fp32)
        LHS_Q = singles.tile([128, T, 2, GG, 4, 16], fp32)
        nc.vector.memset(LHS_L[:, :, :, :, 3, :], 0.0)
        nc.vector.memset(LHS_Q[:, :, :, :, 3, :], 0.0)
        for t in range(T):
            for h in range(2):
                for pi, (lin_c, sq_c) in enumerate(((1, 0), (3, 2), (5, 4))):
                    for dst, ci in ((LHS_L, lin_c), (LHS_Q, sq_c)):
                        csrc = PACKN[:, t, ci, h, :]  # [128, GG]
                        bcs = csrc.unsqueeze(2).broadcast_to([128, GG, 16])
                        h16b = H16.unsqueeze(1).broadcast_to([128, GG, 16])
                        TT(dst[:, t, h, :, pi, :], bcs, h16b, OP.mult)

        # ===== main loop =====
        for t in range(T):
            for c in range(CH):
                ACC = psum_acc.tile([128, 4 * 512], fp32, name="ACC")
                accv = ACC.rearrange("(go pb) (gq n) -> go pb gq n", pb=64, n=512)
                for g in range(GG):
                    xrep = psum_mm.tile([128, 512], fp32, tag="xrep", name="xrep")
                    mmr(xrep, repl[:, g, :, :].rearrange("p a b -> p (a b)"),
                        xc_sb[:, t, c * 512:(c + 1) * 512],
                        start=True, stop=True)
                    slot = ACC[(g % 2) * 64:(g % 2) * 64 + 64,
                               (g // 2) * 512:(g // 2) * 512 + 512]
                    for h in range(2):
                        tk = work.tile([128, 512], fp32, tag="tk", name="tk")
                        TS(tk, xrep, PACKN[:, t, 6, h, g:g + 1],
                           PACKN[:, t, 7, h, g:g + 1], OP.max, OP.min)
                        usq = work.tile([128, 512], fp32, tag="usq", name="usq")
                        nc.scalar.activation(usq, tk, AF.Square,
                                             bias=NEGKX[:, t, h, g:g + 1])
                        if MODE == "u":
                            ulin = work.tile([128, 512], fp32, tag="ulin",
                                             name="ulin")
                            TS(ulin, tk, NEGKX[:, t, h, g:g + 1], None, OP.add)
                            lin_rhs = ulin
                        else:
                            lin_rhs = tk
                        ll = LHS_L[:, t, h, g, :, :].rearrange("p a b -> p (a b)")
                        lq = LHS_Q[:, t, h, g, :, :].rearrange("p a b -> p (a b)")
                        mmr(slot, ll, lin_rhs, start=(h == 0), stop=False)
                        mmr(slot, lq, usq, start=False, stop=(h == 1))

                # PSUM -> SBUF with per-row consts folded in (DMA can't
                # read PSUM); copies split across ACT/DVE
                SACC = fin.tile([128, 4, 512], fp32, tag="SACC", name="SACC")
                for bank in range(4):
                    if bank != 1:
                        nc.scalar.activation(SACC[:, bank, :],
                                             ACC[:, bank * 512:(bank + 1) * 512],
                                             AF.Identity,
                                             bias=CPK[:, t, bank:bank + 1])
                    else:
                        TS(SACC[:, bank, :], ACC[:, bank * 512:(bank + 1) * 512],
                           CPK[:, t, bank:bank + 1], None, OP.add)
                # bounce through DRAM to un-interleave (poly, b) rows:
                # 6 scattered writes + 3 contiguous reads beat 24 direct DMAs
                D1 = dscr.tile([3, 128, 512], fp32, name="D1")
                for go in range(2):
                    for pi in range(3):
                        psrc = SACC[go * 64 + pi * 16:go * 64 + pi * 16 + 16, :, :]
                        dview = bass.AP(
                            tensor=D1.tensor,
                            offset=D1.offset + pi * 128 * 512 + go * 16 * 512,
                            ap=[[512, 16], [32 * 512, 4], [1, 512]])
                        dmax = dma if (go * 3 + pi) % 2 == 0 else nc.scalar.dma_start
                        dmax(out=dview, in_=psrc)
                polys = []
                for pi in range(3):
                    dstt = fin.tile([128, 512], fp32, tag=f"poly{pi}",
                                    name=f"poly{pi}")
                    dmax = dma if pi % 2 == 0 else nc.scalar.dma_start
                    dmax(out=dstt, in_=D1[pi, :, :])
                    polys.append(dstt)
                Np, Dp, Cp = polys

                # finale (ee/inz/outF/ldF on the otherwise-idle GPSIMD)
                ysl = y_sb[:, t, c * 512:(c + 1) * 512]
                xsl = xc_sb[:, t, c * 512:(c + 1) * 512]
                Cm = fin.tile([128, 512], fp32, tag="Cm", name="Cm")
                nc.gpsimd.tensor_scalar(Cm, Cp, 1e-12, None, OP.max)
                rD = fin.tile([128, 512], fp32, tag="rD", name="rD")
                nc.vector.reciprocal(rD, Dp)
                out0 = fin.tile([128, 512], fp32, tag="out0", name="out0")
                nc.gpsimd.tensor_tensor(out0, Np, rD, OP.mult)
                logD = fin.tile([128, 512], fp32, tag="logD", name="logD")
                nc.scalar.activation(logD, Dp, AF.Ln)
                logC = fin.tile([128, 512], fp32, tag="logC", name="logC")
                nc.scalar.activation(logC, Cm, AF.Ln)
                ld0 = fin.tile([128, 512], fp32, tag="ld0", name="ld0")
                STT(ld0, logD, -2.0, logC, OP.mult, OP.add)
                ee = fin.tile([128, 512], fp32, tag="ee", name="ee")
                nc.gpsimd.tensor_tensor(ee, ysl, xsl, OP.subtract)
                inz = fin.tile([128, 512], fp32, tag="inz", name="inz")
                nc.gpsimd.tensor_scalar(inz, ee, 0.0, None, OP.is_equal)
                outF = fin.tile([128, 512], fp32, tag="outF", name="outF")
                nc.gpsimd.tensor_tensor(outF, out0, ee, OP.add)
                ldF = fin.tile([128, 512], fp32, tag="ldF", name="ldF")
                nc.gpsimd.tensor_tensor(ldF, ld0, inz, OP.mult)
                dma(out=io["out"][t * 128:(t + 1) * 128, c * 512:(c + 1) * 512],
                    in_=outF)
                nc.scalar.dma_start(
                    out=io["logdet"][t * 128:(t + 1) * 128, c * 512:(c + 1) * 512],
                    in_=ldF)


def kernel(cond, y, W1, b1, W2, b2, W3, b3):
    _ensure_path()
    from concourse.bass_utils import run_bass_kernel_spmd

    if "nc" not in _CACHE:
        _CACHE["nc"] = _build_nc()
    nc = _CACHE["nc"]

    cond = np.ascontiguousarray(cond, np.float32)
    y = np.ascontiguousarray(y, np.float32)
    shared = dict(W1=np.ascontiguousarray(W1, np.float32),
                  b1=np.ascontiguousarray(b1, np.float32),
                  W2=np.ascontiguousarray(W2, np.float32),
                  b2=np.ascontiguousarray(b2, np.float32),
                  W3=np.ascontiguousarray(W3, np.float32),
                  b3=np.ascontiguousarray(b3, np.float32))
    in_maps = []
    for i in range(NCORES):
        sl = slice(i * BL, (i + 1) * BL)
        in_maps.append(dict(cond=cond[sl], y=y[sl], **shared))
    res = run_bass_kernel_spmd(nc, in_maps, core_ids=list(range(NCORES)))
    out = np.concatenate([r["out"] for r in res.results], axis=0)
    ld = np.concatenate([r["logdet"] for r in res.results], axis=0)
    return out, ld



# revision 14
# speedup vs baseline: 1.5498x; 1.0147x over previous
"""Trainium2 Bass kernel for nn_ConditionalSpline1DFlow (K=16 RQS flow).

Data-parallel over 8 cores (B=4096 -> 512 rows/core). Per core:
  1. Conditioner MLP on TensorE (feature-major).
  2. Spline params per row; rescale bin k's rational-quadratic by
     s_k = delta_0/delta_k so numerator N, denominator D and
     derivative-numerator C become globally CONTINUOUS piecewise
     quadratics in x.
  3. Evaluate N, D, C gather-free in the clipped-ramp basis
        P(x) = const + sum_k a_k*(t_k - x_k)^2 + b_k*(t_k - x_k),
        t_k = clip(x, x_k, x_{k+1})
     on TensorE: rows packed (b*16+k) so one [128, 24] matmul contracts
     all 16 bins x 3 polys for 8 batch rows at once; PSUM accumulates the
     (linear, square) stream pair.
  4. out = N/D + (y - clip(y)); logdet = (ln C - 2 ln D) * (y == clip(y)).
"""
import sys
import numpy as np

K = 16
BOUND = 5.0
MBW = 1e-3
MBH = 1e-3
MD = 1e-3
B_FULL, N = 4096, 1024
CD, H = 64, 256
OUT3 = 3 * K + 1
NCORES = 8
BL = B_FULL // NCORES   # 512 rows per core
T = BL // 128           # 4 partition tiles
G = 128 // 8            # (unused) 8-row groups
GG = 128 // 16          # 8 groups of 16 rows per tile
CH = N // 512           # 2 free-dim chunks

MODE = "t"  # "t": stream clipped-t w/ folded consts; "u": stream t - x_k

_CACHE = {}


def _ensure_path():
    for p in ("/opt/trn_rl_repo",):
        if p not in sys.path:
            sys.path.insert(0, p)


def _build_nc():
    _ensure_path()
    import concourse.bacc as bacc
    import concourse.tile as tile
    from concourse import mybir

    fp32 = mybir.dt.float32
    nc = bacc.Bacc("TRN2", target_bir_lowering=False, debug=False)

    io = dict(
        cond=nc.dram_tensor("cond", [BL, CD], fp32, kind="ExternalInput"),
        y=nc.dram_tensor("y", [BL, N], fp32, kind="ExternalInput"),
        W1=nc.dram_tensor("W1", [CD, H], fp32, kind="ExternalInput"),
        b1=nc.dram_tensor("b1", [H], fp32, kind="ExternalInput"),
        W2=nc.dram_tensor("W2", [H, H], fp32, kind="ExternalInput"),
        b2=nc.dram_tensor("b2", [H], fp32, kind="ExternalInput"),
        W3=nc.dram_tensor("W3", [H, OUT3], fp32, kind="ExternalInput"),
        b3=nc.dram_tensor("b3", [OUT3], fp32, kind="ExternalInput"),
        out=nc.dram_tensor("out", [BL, N], fp32, kind="ExternalOutput"),
        logdet=nc.dram_tensor("logdet", [BL, N], fp32, kind="ExternalOutput"),
    )
    with tile.TileContext(nc) as tc:
        _emit(nc, tc, io)
    nc.compile()
    return nc


def _emit(nc, tc, io):
    from contextlib import ExitStack
    import concourse.bass as bass
    from concourse import mybir

    fp32 = mybir.dt.float32
    i32 = mybir.dt.int32
    AF = mybir.ActivationFunctionType
    OP = mybir.AluOpType
    AX = mybir.AxisListType

    TT = nc.vector.tensor_tensor
    TS = nc.vector.tensor_scalar
    STT = nc.vector.scalar_tensor_tensor
    fp32r = mybir.dt.float32r

    def mmr(out, lhsT, rhs, **kw):
        # fp32r (TF32-like): 4x faster on PE when the moving free dim >= 256.
        # Rel-err budget is 2e-2; TF32 rounding costs ~1e-3 — well inside.
        # Both operands must come from fp32r-producing instructions (walrus
        # BIR verifier rule), so their tiles are allocated as fp32r below.
        nc.tensor.matmul(out, lhsT, rhs, **kw)

    ctx = ExitStack()
    with ctx:
        singles = ctx.enter_context(tc.tile_pool(name="singles", bufs=1))
        work = ctx.enter_context(tc.tile_pool(name="work", bufs=3))
        fin = ctx.enter_context(tc.tile_pool(name="fin", bufs=2))
        psum_mm = ctx.enter_context(tc.tile_pool(name="psum_mm", bufs=2, space="PSUM"))
        psum_acc = ctx.enter_context(tc.tile_pool(name="psum_acc", bufs=1, space="PSUM"))
        dscr = ctx.enter_context(tc.tile_pool(name="dscr", bufs=2, space="DRAM"))

        dma = nc.sync.dma_start

        cnt = [0]

        def ps_tile(p, f):
            cnt[0] += 1
            return psum_mm.tile([p, f], fp32, tag="ps", name=f"ps{cnt[0]}")

        # ===== iota-derived constant masks =====
        iota_i = singles.tile([128, 1], i32)
        nc.gpsimd.iota(iota_i, pattern=[[0, 1]], base=0, channel_multiplier=1)
        iota_f = singles.tile([128, 1], fp32)
        nc.vector.tensor_copy(iota_f, iota_i)

        bkf_i = singles.tile([128, 16, 8], i32)   # value b' at col (b'*8+m)
        nc.gpsimd.iota(bkf_i, pattern=[[1, 16], [0, 8]], base=0, channel_multiplier=0)
        bkf_f = singles.tile([128, 16, 8], fp32)
        nc.vector.tensor_copy(bkf_f, bkf_i)

        colf_i = singles.tile([128, 128], i32)    # value j at col j
        nc.gpsimd.iota(colf_i, pattern=[[1, 128]], base=0, channel_multiplier=0)
        colf_f = singles.tile([128, 128], fp32)
        nc.vector.tensor_copy(colf_f, colf_i)

        pmod_i = singles.tile([128, 1], i32)      # p % 16
        TS(pmod_i, iota_i, 15, None, OP.bitwise_and)
        pmod_f = singles.tile([128, 1], fp32)
        nc.vector.tensor_copy(pmod_f, pmod_i)

        ident = singles.tile([128, 128], fp32)    # identity matrix
        TS(ident, colf_f, iota_f, None, OP.is_equal)

        lhsT16 = singles.tile([16, 128], fp32)     # [b, b'*8+m] = (b'==b)
        TS(lhsT16, bkf_f.rearrange("p a b -> p (a b)")[:16], iota_f[:16], None,
           OP.is_equal)

        maskbb = singles.tile([128, 16, 8], fp32)  # [p, (b',m)] = (p%16==b')
        TS(maskbb, bkf_f, pmod_f, None, OP.is_equal)

        # per-group replication masks: repl[gg][p, (b',m)] = (p == 16gg+b')
        # fp32r: feeds the fp32r xrep matmul (0/1 values, rounding exact)
        repl = singles.tile([128, GG, 16, 8], fp32r)
        for g in range(GG):
            pg = work.tile([128, 1], fp32, tag="pg", name="pg")
            TS(pg, iota_f, float(-16 * g), None, OP.add)
            TS(repl[:, g, :, :], bkf_f, pg, None, OP.is_equal)

        ps_h16 = ps_tile(128, 16)
        nc.tensor.transpose(ps_h16, lhsT16, ident[:16, :16])
        H16 = singles.tile([128, 16], fp32)        # [p, b'] = (p//8==b')
        nc.scalar.copy(H16, ps_h16)

        # gsel[p, g] = (p//16 == g); gqsel[p, q] = (p//32 == q)
        pdiv16_i = singles.tile([128, 1], i32)
        TS(pdiv16_i, iota_i, 4, None, OP.arith_shift_right)
        pdiv16_f = singles.tile([128, 1], fp32)
        nc.vector.tensor_copy(pdiv16_f, pdiv16_i)
        col8_i = singles.tile([128, 8], i32)
        nc.gpsimd.iota(col8_i, pattern=[[1, 8]], base=0, channel_multiplier=0)
        col8_f = singles.tile([128, 8], fp32)
        nc.vector.tensor_copy(col8_f, col8_i)
        gsel = singles.tile([128, 8], fp32)
        TS(gsel, col8_f, pdiv16_f, None, OP.is_equal)

        pdiv32_i = singles.tile([128, 1], i32)
        TS(pdiv32_i, iota_i, 5, None, OP.arith_shift_right)
        pdiv32_f = singles.tile([128, 1], fp32)
        nc.vector.tensor_copy(pdiv32_f, pdiv32_i)
        gqsel = singles.tile([128, 4], fp32)
        TS(gqsel, col8_f[:, 0:4], pdiv32_f, None, OP.is_equal)

        # maskC[p, (go',pi',b')] = ((p//16)%2 == go') * (p%16 == b')
        pm2_i = singles.tile([128, 1], i32)
        TS(pm2_i, pdiv16_i, 1, None, OP.bitwise_and)
        pm2_f = singles.tile([128, 1], fp32)
        nc.vector.tensor_copy(pm2_f, pm2_i)
        gof_i = singles.tile([128, 2, 4, 16], i32)
        nc.gpsimd.iota(gof_i, pattern=[[1, 2], [0, 4], [0, 16]], base=0,
                       channel_multiplier=0)
        gof_f = singles.tile([128, 2, 4, 16], fp32)
        nc.vector.tensor_copy(gof_f, gof_i)
        bf2_i = singles.tile([128, 2, 4, 16], i32)
        nc.gpsimd.iota(bf2_i, pattern=[[0, 2], [0, 4], [1, 16]], base=0,
                       channel_multiplier=0)
        bf2_f = singles.tile([128, 2, 4, 16], fp32)
        nc.vector.tensor_copy(bf2_f, bf2_i)
        mgo = singles.tile([128, 2, 4, 16], fp32)
        TS(mgo, gof_f, pm2_f, None, OP.is_equal)
        maskC = singles.tile([128, 2, 4, 16], fp32)
        mb2 = singles.tile([128, 2, 4, 16], fp32)
        TS(mb2, bf2_f, pmod_f, None, OP.is_equal)
        TT(maskC, mgo, mb2, OP.mult)

        # ===== weights =====
        # DMA exact fp32, then round to fp32r copies for the fp32r matmuls
        W1d = singles.tile([CD, H], fp32)
        dma(out=W1d, in_=io["W1"][:, :])
        W2d = [singles.tile([128, H], fp32, tag=f"w2d_{i}", name=f"w2d_{i}") for i in range(2)]
        W3d = [singles.tile([128, OUT3], fp32, tag=f"w3d_{i}", name=f"w3d_{i}") for i in range(2)]
        for i in range(2):
            dma(out=W2d[i], in_=io["W2"][i * 128:(i + 1) * 128, :])
            dma(out=W3d[i], in_=io["W3"][i * 128:(i + 1) * 128, :])
        W1s = singles.tile([CD, H], fp32r)
        nc.scalar.copy(W1s, W1d)
        W2s = [singles.tile([128, H], fp32r, tag=f"w2_{i}", name=f"w2_{i}") for i in range(2)]
        W3s = [singles.tile([128, OUT3], fp32r, tag=f"w3_{i}", name=f"w3_{i}") for i in range(2)]
        for i in range(2):
            nc.scalar.copy(W2s[i], W2d[i])
            nc.scalar.copy(W3s[i], W3d[i])
        b1t = singles.tile([128, 2], fp32)
        dma(out=b1t, in_=io["b1"].rearrange("(h p) -> p h", p=128))
        b2t = singles.tile([128, 2], fp32)
        dma(out=b2t, in_=io["b2"].rearrange("(h p) -> p h", p=128))
        b3t = singles.tile([OUT3, 1], fp32)
        dma(out=b3t, in_=io["b3"].rearrange("(o u) -> o u", u=1))

        # ===== y, xc =====
        # Only the fp32r-rounded clip is stored (feeds the PE matmuls); the
        # exact clip for the outside-mask is recomputed per chunk in the
        # finale, saving 16KB/partition of SBUF.
        y_sb = singles.tile([128, T, N], fp32)
        xc_r = singles.tile([128, T, N], fp32r)    # rounded: feeds PE matmul
        for t in range(T):
            dma(out=y_sb[:, t, :], in_=io["y"][t * 128:(t + 1) * 128, :])
        for t in range(T):
            nc.gpsimd.tensor_scalar(xc_r[:, t, :], y_sb[:, t, :], -BOUND, BOUND,
                                    OP.max, OP.min)

        # ===== MLP =====
        condT = singles.tile([CD, BL], fp32r)
        for t in range(T):
            csb = work.tile([128, CD], fp32, tag="cond", name="csb")
            dma(out=csb, in_=io["cond"][t * 128:(t + 1) * 128, :])
            ps = ps_tile(CD, 128)
            nc.tensor.transpose(ps, csb, ident)
            nc.scalar.copy(condT[:, t * 128:(t + 1) * 128], ps)

        h1 = [singles.tile([128, BL], fp32r, tag=f"h1_{i}", name=f"h1_{i}") for i in range(2)]
        for half in range(2):
            ps = ps_tile(128, BL)
            mmr(ps, W1s[:, half * 128:(half + 1) * 128], condT,
                start=True, stop=True)
            nc.scalar.activation(h1[half], ps, AF.Relu, bias=b1t[:, half:half + 1])
        h2 = [singles.tile([128, BL], fp32r, tag=f"h2_{i}", name=f"h2_{i}") for i in range(2)]
        for half in range(2):
            ps = ps_tile(128, BL)
            for kc in range(2):
                mmr(ps, W2s[kc][:, half * 128:(half + 1) * 128], h1[kc],
                    start=(kc == 0), stop=(kc == 1))
            nc.scalar.activation(h2[half], ps, AF.Relu, bias=b2t[:, half:half + 1])
        p_f = singles.tile([OUT3, BL], fp32)
        ps49 = ps_tile(OUT3, BL)
        for kc in range(2):
            mmr(ps49, W3s[kc], h2[kc], start=(kc == 0), stop=(kc == 1))
        nc.scalar.activation(p_f, ps49, AF.Identity, bias=b3t)

        pw = singles.tile([128, T, OUT3], fp32)   # p row-major
        for t in range(T):
            ps = ps_tile(128, OUT3)
            nc.tensor.transpose(ps, p_f[:, t * 128:(t + 1) * 128], ident[:OUT3, :OUT3])
            nc.scalar.copy(pw[:, t, :], ps)

        # ===== param pipeline =====
        un_w = pw[:, :, 0:K]
        un_h = pw[:, :, K:2 * K]
        un_d = pw[:, :, 2 * K:3 * K + 1]

        def softmax_w(un, mb, tag):
            mx = singles.tile([128, T], fp32, tag=f"mx{tag}", name=f"mx{tag}")
            nc.vector.tensor_reduce(mx, un, axis=AX.X, op=OP.max)
            nmx = singles.tile([128, T], fp32, tag=f"nmx{tag}", name=f"nmx{tag}")
            TS(nmx, mx, -1.0, None, OP.mult)
            ein = singles.tile([128, T, K], fp32, tag=f"ein{tag}", name=f"ein{tag}")
            for t in range(T):
                TS(ein[:, t, :], un[:, t, :], nmx[:, t:t + 1], None, OP.add)
            ex = singles.tile([128, T, K], fp32, tag=f"ex{tag}", name=f"ex{tag}")
            nc.scalar.activation(ex, ein, AF.Exp)
            sm = singles.tile([128, T], fp32, tag=f"sm{tag}", name=f"sm{tag}")
            nc.vector.tensor_reduce(sm, ex, axis=AX.X, op=OP.add)
            rs = singles.tile([128, T], fp32, tag=f"rs{tag}", name=f"rs{tag}")
            nc.vector.reciprocal(rs, sm)
            wd = singles.tile([128, T, K], fp32, tag=f"wd{tag}", name=f"wd{tag}")
            for t in range(T):
                TS(wd[:, t, :], ex[:, t, :], rs[:, t:t + 1], 2 * BOUND - K * mb,
                   OP.mult, OP.mult)
            TS(wd, wd, mb, None, OP.add)
            return wd

        widths = softmax_w(un_w, MBW, "w")
        heights = softmax_w(un_h, MBH, "h")

        zeros16 = singles.tile([128, K], fp32)
        nc.vector.memset(zeros16, 0.0)
        cumw = singles.tile([128, T, K + 1], fp32)
        cumh = singles.tile([128, T, K + 1], fp32)
        nc.vector.memset(cumw[:, :, 0:1], -BOUND)
        nc.vector.memset(cumh[:, :, 0:1], -BOUND)
        for t in range(T):
            nc.vector.tensor_tensor_scan(cumw[:, t, 1:], widths[:, t, :], zeros16,
                                         -BOUND, OP.add, OP.add)
            nc.vector.tensor_tensor_scan(cumh[:, t, 1:], heights[:, t, :], zeros16,
                                         -BOUND, OP.add, OP.add)

        # softplus(x) = max(x,0) + ln(1 + exp(-|x|)) (no Softplus table on TRN2)
        deriv = singles.tile([128, T, K + 1], fp32)
        absd = singles.tile([128, T, K + 1], fp32)
        nc.scalar.activation(absd, un_d, AF.Abs)
        end_ = singles.tile([128, T, K + 1], fp32)
        nc.scalar.activation(end_, absd, AF.Exp, scale=-1.0)
        l1p = singles.tile([128, T, K + 1], fp32)
        nc.scalar.activation(l1p, end_, AF.Ln, bias=1.0)
        rl = singles.tile([128, T, K + 1], fp32)
        TS(rl, un_d, 0.0, MD, OP.max, OP.add)
        TT(deriv, rl, l1p, OP.add)

        d0 = deriv[:, :, 0:K]
        d1 = deriv[:, :, 1:K + 1]
        y0 = cumh[:, :, 0:K]
        kx = cumw[:, :, 0:K]
        kx1 = cumw[:, :, 1:K + 1]

        def tmp(tag):
            return singles.tile([128, T, K], fp32, tag=tag, name=tag)

        iw = tmp("iw"); nc.vector.reciprocal(iw, widths)
        delta = tmp("delta"); TT(delta, heights, iw, OP.mult)
        rdelta = tmp("rdelta"); nc.vector.reciprocal(rdelta, delta)
        s = tmp("s")
        for t in range(T):
            TS(s[:, t, :], rdelta[:, t, :], delta[:, t, 0:1], None, OP.mult)
        sig = tmp("sig"); TT(sig, d0, d1, OP.add)
        STT(sig, delta, -2.0, sig, OP.mult, OP.add)
        sdelta = tmp("sdelta"); TT(sdelta, s, delta, OP.mult)
        ssig = tmp("ssig"); TT(ssig, s, sig, OP.mult)
        sh = tmp("sh"); TT(sh, s, heights, OP.mult)
        shd0 = tmp("shd0"); TT(shd0, sh, d0, OP.mult)
        t1 = tmp("t1"); TT(t1, y0, ssig, OP.mult)
        Nc1 = tmp("Nc1"); TT(Nc1, t1, shd0, OP.add)
        u1 = tmp("u1"); TT(u1, delta, d0, OP.subtract)
        u2 = tmp("u2"); TT(u2, sh, u1, OP.mult)
        Nc2 = tmp("Nc2"); TT(Nc2, u2, t1, OP.subtract)
        sd2 = tmp("sd2"); TT(sd2, sdelta, sdelta, OP.mult)
        Cc1 = tmp("Cc1"); STT(Cc1, sd2, 2.0, u1, OP.mult, OP.mult)
        Cc2 = tmp("Cc2"); TT(Cc2, sd2, sig, OP.mult)
        iw2 = tmp("iw2"); TT(iw2, iw, iw, OP.mult)

        # final coefs into one contiguous tile: coefcat[:, t, ci, k]
        # ci: 0=aN 1=bN 2=aD 3=bD 4=aC 5=bC 6=kx 7=kx1
        coefcat = singles.tile([128, T, 8, K], fp32)
        aN = coefcat[:, :, 0, :]; TT(aN, Nc2, iw2, OP.mult)
        bN = coefcat[:, :, 1, :]; TT(bN, Nc1, iw, OP.mult)
        aD = coefcat[:, :, 2, :]; STT(aD, ssig, -1.0, iw2, OP.mult, OP.mult)
        bD = coefcat[:, :, 3, :]; TT(bD, ssig, iw, OP.mult)
        aC = coefcat[:, :, 4, :]; TT(aC, Cc2, iw2, OP.mult)
        bC = coefcat[:, :, 5, :]; TT(bC, Cc1, iw, OP.mult)
        nc.vector.tensor_copy(coefcat[:, :, 6, :], kx)
        nc.vector.tensor_copy(coefcat[:, :, 7, :], kx1)

        # per-row constants, packed 4-wide (pi 3 = 0) for the cpk transform
        constcat4 = singles.tile([128, 4, T], fp32)
        nc.vector.memset(constcat4[:, 3, :], 0.0)
        constN = constcat4[:, 0, :]
        TT(constN, y0[:, :, 0], sdelta[:, :, 0], OP.mult)
        constD = constcat4[:, 1, :]
        nc.vector.tensor_copy(constD, sdelta[:, :, 0])
        constC = constcat4[:, 2, :]
        TT(constC, sd2[:, :, 0], d0[:, :, 0], OP.mult)

        if MODE == "t":
            for cst, b in ((constN, bN), (constD, bD), (constC, bC)):
                bx = tmp("bx"); TT(bx, b, kx, OP.mult)
                sbx = singles.tile([128, T], fp32, tag="sbx", name="sbx")
                nc.vector.tensor_reduce(sbx, bx, axis=AX.X, op=OP.add)
                TT(cst, cst, sbx, OP.subtract)

        # ===== repack coefficients to (b*8+m) partition layout, k = 8h+m ====
        # PACKN[p=(b*8+m), t, ci, h, g] = coefcat[16g+b, t, ci, 8h+m]
        # via PE: PACK = (coef-expand * maskbb)^T @ gsel  (contraction over
        # the 128 source rows; gsel selects the group).
        PACKN = singles.tile([128, T, 8, 2, GG], fp32)
        for t in range(T):
            psp = ps_tile(128, 128)
            for h in range(2):
                exbig = work.tile([128, 8, 16, 8], fp32, tag="exbig",
                                  name="exbig")
                in0 = coefcat[:, t, :, 8 * h:8 * h + 8].unsqueeze(2)\
                    .broadcast_to([128, 8, 16, 8])
                in1 = maskbb.unsqueeze(1).broadcast_to([128, 8, 16, 8])
                TT(exbig, in0, in1, OP.mult)
                for ci in range(8):
                    lhs = exbig[:, ci, :, :].rearrange("p a b -> p (a b)")
                    nc.tensor.matmul(psp[:, (ci * 2 + h) * 8:(ci * 2 + h) * 8 + 8],
                                     lhs, gsel, start=True, stop=True)
            nc.scalar.copy(
                PACKN[:, t, :, :, :].rearrange("p a b c -> p (a b c)"), psp)
        NEGKX = singles.tile([128, T, 2, GG], fp32)
        TS(NEGKX, PACKN[:, :, 6, :, :], -1.0, None, OP.mult)

        # cpk[p=(go*64+pi*16+b), t, gq] = const_pi[32gq+16go+b, t] (SACC layout)
        CPK = singles.tile([128, T, 4], fp32)
        psc = ps_tile(128, 16)
        for t in range(T):
            cE = work.tile([128, 2, 4, 16], fp32, tag="cE", name="cE")
            cin = constcat4[:, :, t].unsqueeze(1).unsqueeze(3)\
                .broadcast_to([128, 2, 4, 16])
            TT(cE, cin, maskC, OP.mult)
            nc.tensor.matmul(psc[:, t * 4:(t + 1) * 4],
                             cE.rearrange("p a b c -> p (a b c)"), gqsel,
                             start=True, stop=True)
        nc.scalar.copy(CPK.rearrange("p a b -> p (a b)"), psc)

        # lhsT mega: [128, T, 2, GG, 4, 16]; per (t,h,g) a contiguous
        # [4poly, 16b'] = 64-col block (poly 3 = zeros, pads po to 64 so
        # start=True initializes the full PSUM slot)
        LHS_L = singles.tile([128, T, 2, GG, 4, 16], fp32r)
        LHS_Q = singles.tile([128, T, 2, GG, 4, 16], fp32r)
        nc.vector.memset(LHS_L[:, :, :, :, 3, :], 0.0)
        nc.vector.memset(LHS_Q[:, :, :, :, 3, :], 0.0)
        for t in range(T):
            for h in range(2):
                for pi, (lin_c, sq_c) in enumerate(((1, 0), (3, 2), (5, 4))):
                    for dst, ci in ((LHS_L, lin_c), (LHS_Q, sq_c)):
                        csrc = PACKN[:, t, ci, h, :]  # [128, GG]
                        bcs = csrc.unsqueeze(2).broadcast_to([128, GG, 16])
                        h16b = H16.unsqueeze(1).broadcast_to([128, GG, 16])
                        TT(dst[:, t, h, :, pi, :], bcs, h16b, OP.mult)

        # ===== main loop =====
        for t in range(T):
            for c in range(CH):
                ACC = psum_acc.tile([128, 4 * 512], fp32, name="ACC")
                accv = ACC.rearrange("(go pb) (gq n) -> go pb gq n", pb=64, n=512)
                for g in range(GG):
                    xrep = psum_mm.tile([128, 512], fp32, tag="xrep", name="xrep")
                    mmr(xrep, repl[:, g, :, :].rearrange("p a b -> p (a b)"),
                        xc_r[:, t, c * 512:(c + 1) * 512],
                        start=True, stop=True)
                    slot = ACC[(g % 2) * 64:(g % 2) * 64 + 64,
                               (g // 2) * 512:(g // 2) * 512 + 512]
                    for h in range(2):
                        tk = work.tile([128, 512], fp32r, tag="tk", name="tk")
                        TS(tk, xrep, PACKN[:, t, 6, h, g:g + 1],
                           PACKN[:, t, 7, h, g:g + 1], OP.max, OP.min)
                        usq = work.tile([128, 512], fp32r, tag="usq", name="usq")
                        nc.scalar.activation(usq, tk, AF.Square,
                                             bias=NEGKX[:, t, h, g:g + 1])
                        if MODE == "u":
                            ulin = work.tile([128, 512], fp32, tag="ulin",
                                             name="ulin")
                            TS(ulin, tk, NEGKX[:, t, h, g:g + 1], None, OP.add)
                            lin_rhs = ulin
                        else:
                            lin_rhs = tk
                        ll = LHS_L[:, t, h, g, :, :].rearrange("p a b -> p (a b)")
                        lq = LHS_Q[:, t, h, g, :, :].rearrange("p a b -> p (a b)")
                        mmr(slot, ll, lin_rhs, start=(h == 0), stop=False)
                        mmr(slot, lq, usq, start=False, stop=(h == 1))

                # PSUM -> SBUF with per-row consts folded in (DMA can't
                # read PSUM); copies split across ACT/DVE
                SACC = fin.tile([128, 4, 512], fp32, tag="SACC", name="SACC")
                for bank in range(4):
                    if bank != 1:
                        nc.scalar.activation(SACC[:, bank, :],
                                             ACC[:, bank * 512:(bank + 1) * 512],
                                             AF.Identity,
                                             bias=CPK[:, t, bank:bank + 1])
                    else:
                        TS(SACC[:, bank, :], ACC[:, bank * 512:(bank + 1) * 512],
                           CPK[:, t, bank:bank + 1], None, OP.add)
                # bounce through DRAM to un-interleave (poly, b) rows:
                # 6 scattered writes + 3 contiguous reads beat 24 direct DMAs
                D1 = dscr.tile([3, 128, 512], fp32, name="D1")
                for go in range(2):
                    for pi in range(3):
                        psrc = SACC[go * 64 + pi * 16:go * 64 + pi * 16 + 16, :, :]
                        dview = bass.AP(
                            tensor=D1.tensor,
                            offset=D1.offset + pi * 128 * 512 + go * 16 * 512,
                            ap=[[512, 16], [32 * 512, 4], [1, 512]])
                        dmax = dma if (go * 3 + pi) % 2 == 0 else nc.scalar.dma_start
                        dmax(out=dview, in_=psrc)
                polys = []
                for pi in range(3):
                    dstt = fin.tile([128, 512], fp32, tag=f"poly{pi}",
                                    name=f"poly{pi}")
                    dmax = dma if pi % 2 == 0 else nc.scalar.dma_start
                    dmax(out=dstt, in_=D1[pi, :, :])
                    polys.append(dstt)
                Np, Dp, Cp = polys

                # finale (ee/inz/outF/ldF on the otherwise-idle GPSIMD)
                ysl = y_sb[:, t, c * 512:(c + 1) * 512]
                xsl = fin.tile([128, 512], fp32, tag="xsl", name="xsl")
                nc.gpsimd.tensor_scalar(xsl, ysl, -BOUND, BOUND, OP.max, OP.min)
                Cm = fin.tile([128, 512], fp32, tag="Cm", name="Cm")
                nc.gpsimd.tensor_scalar(Cm, Cp, 1e-12, None, OP.max)
                rD = fin.tile([128, 512], fp32, tag="rD", name="rD")
                nc.vector.reciprocal(rD, Dp)
                out0 = fin.tile([128, 512], fp32, tag="out0", name="out0")
                nc.gpsimd.tensor_tensor(out0, Np, rD, OP.mult)
                logD = fin.tile([128, 512], fp32, tag="logD", name="logD")
                nc.scalar.activation(logD, Dp, AF.Ln)
                logC = fin.tile([128, 512], fp32, tag="logC", name="logC")
                nc.scalar.activation(logC, Cm, AF.Ln)
                ld0 = fin.tile([128, 512], fp32, tag="ld0", name="ld0")
                STT(ld0, logD, -2.0, logC, OP.mult, OP.add)
                ee = fin.tile([128, 512], fp32, tag="ee", name="ee")
                nc.gpsimd.tensor_tensor(ee, ysl, xsl, OP.subtract)
                inz = fin.tile([128, 512], fp32, tag="inz", name="inz")
                nc.gpsimd.tensor_scalar(inz, ee, 0.0, None, OP.is_equal)
                outF = fin.tile([128, 512], fp32, tag="outF", name="outF")
                nc.gpsimd.tensor_tensor(outF, out0, ee, OP.add)
                ldF = fin.tile([128, 512], fp32, tag="ldF", name="ldF")
                nc.gpsimd.tensor_tensor(ldF, ld0, inz, OP.mult)
                dma(out=io["out"][t * 128:(t + 1) * 128, c * 512:(c + 1) * 512],
                    in_=outF)
                nc.scalar.dma_start(
                    out=io["logdet"][t * 128:(t + 1) * 128, c * 512:(c + 1) * 512],
                    in_=ldF)


def kernel(cond, y, W1, b1, W2, b2, W3, b3):
    _ensure_path()
    from concourse.bass_utils import run_bass_kernel_spmd

    if "nc" not in _CACHE:
        _CACHE["nc"] = _build_nc()
    nc = _CACHE["nc"]

    cond = np.ascontiguousarray(cond, np.float32)
    y = np.ascontiguousarray(y, np.float32)
    shared = dict(W1=np.ascontiguousarray(W1, np.float32),
                  b1=np.ascontiguousarray(b1, np.float32),
                  W2=np.ascontiguousarray(W2, np.float32),
                  b2=np.ascontiguousarray(b2, np.float32),
                  W3=np.ascontiguousarray(W3, np.float32),
                  b3=np.ascontiguousarray(b3, np.float32))
    in_maps = []
    for i in range(NCORES):
        sl = slice(i * BL, (i + 1) * BL)
        in_maps.append(dict(cond=cond[sl], y=y[sl], **shared))
    res = run_bass_kernel_spmd(nc, in_maps, core_ids=list(range(NCORES)))
    out = np.concatenate([r["out"] for r in res.results], axis=0)
    ld = np.concatenate([r["logdet"] for r in res.results], axis=0)
    return out, ld



# revision 16
# speedup vs baseline: 1.6804x; 1.0843x over previous
"""Trainium2 Bass kernel for nn_ConditionalSpline1DFlow (K=16 RQS flow).

Data-parallel over 8 cores (B=4096 -> 512 rows/core). Per core:
  1. Conditioner MLP on TensorE (feature-major, fp32r matmuls).
  2. Spline params per row; rescale bin k's rational-quadratic by
     s_k = delta_0/delta_k so numerator N, denominator D and
     derivative-numerator C become globally CONTINUOUS piecewise
     quadratics in x.
  3. Evaluate N, D, C gather-free in the clipped-ramp basis
        P(x) = const + sum_k a_k*t_k^2' + b_k*t_k (consts folded),
        t_k = clip(x, x_k, x_{k+1})
     on TensorE with fp32r (1 cycle/row): partitions p = 32*m + b
     hold lane-bin m (bin k = 4m+h over 4 h-passes) for batch row b
     (32 rows/Q-block); x replicated 4x by DMA broadcast from DRAM.
     One [128, 96=(3 poly x 32 b)] stationary per (Q,s,h); all matmul
     dsts are base-partition-0 (fp32r ISA requirement).
  4. out = N/D + (y - clip(y)); logdet = (ln C - 2 ln D) * (y == clip(y)).
"""
import sys
import numpy as np

K = 16
BOUND = 5.0
MBW = 1e-3
MBH = 1e-3
MD = 1e-3
B_FULL, N = 4096, 1024
CD, H = 64, 256
OUT3 = 3 * K + 1
NCORES = 8
BL = B_FULL // NCORES   # 512 rows per core
T = BL // 128           # 4 partition tiles
CH = N // 512           # 2 free-dim chunks
QQ = 4                  # 32-row blocks per tile
HH = 4                  # h-passes (bins k = 4m+h)

_CACHE = {}


def _ensure_path():
    for p in ("/opt/trn_rl_repo",):
        if p not in sys.path:
            sys.path.insert(0, p)


def _build_nc():
    _ensure_path()
    import concourse.bacc as bacc
    import concourse.tile as tile
    from concourse import mybir

    fp32 = mybir.dt.float32
    nc = bacc.Bacc("TRN2", target_bir_lowering=False, debug=False)

    io = dict(
        cond=nc.dram_tensor("cond", [BL, CD], fp32, kind="ExternalInput"),
        y=nc.dram_tensor("y", [BL, N], fp32, kind="ExternalInput"),
        W1=nc.dram_tensor("W1", [CD, H], fp32, kind="ExternalInput"),
        b1=nc.dram_tensor("b1", [H], fp32, kind="ExternalInput"),
        W2=nc.dram_tensor("W2", [H, H], fp32, kind="ExternalInput"),
        b2=nc.dram_tensor("b2", [H], fp32, kind="ExternalInput"),
        W3=nc.dram_tensor("W3", [H, OUT3], fp32, kind="ExternalInput"),
        b3=nc.dram_tensor("b3", [OUT3], fp32, kind="ExternalInput"),
        out=nc.dram_tensor("out", [BL, N], fp32, kind="ExternalOutput"),
        logdet=nc.dram_tensor("logdet", [BL, N], fp32, kind="ExternalOutput"),
    )
    with tile.TileContext(nc) as tc:
        _emit(nc, tc, io)
    nc.compile()
    return nc


def _emit(nc, tc, io):
    from contextlib import ExitStack
    import concourse.bass as bass
    from concourse import mybir

    fp32 = mybir.dt.float32
    fp32r = mybir.dt.float32r
    i32 = mybir.dt.int32
    AF = mybir.ActivationFunctionType
    OP = mybir.AluOpType
    AX = mybir.AxisListType

    TT = nc.vector.tensor_tensor
    TS = nc.vector.tensor_scalar
    STT = nc.vector.scalar_tensor_tensor

    def mmr(out, lhsT, rhs, **kw):
        # fp32r (TF32-like): 1 cycle/row on PE when moving free dim >= 256
        # (vs 4 for fp32). Both operands produced as fp32r; dst base
        # partition must be 0 (ISA).
        nc.tensor.matmul(out, lhsT, rhs, **kw)

    ctx = ExitStack()
    with ctx:
        singles = ctx.enter_context(tc.tile_pool(name="singles", bufs=1))
        work = ctx.enter_context(tc.tile_pool(name="work", bufs=3))
        lhsp = ctx.enter_context(tc.tile_pool(name="lhsp", bufs=2))
        fin = ctx.enter_context(tc.tile_pool(name="fin", bufs=2))
        dscr = ctx.enter_context(tc.tile_pool(name="dscr", bufs=1, space="DRAM"))

        dma = nc.sync.dma_start

        # ================= setup phase (uses its own PSUM pool) ========
        with tc.tile_pool(name="ps_setup", bufs=2, space="PSUM") as pssetup:
            cnt = [0]

            def ps_tile(p, f):
                cnt[0] += 1
                return pssetup.tile([p, f], fp32, tag="ps", name=f"ps{cnt[0]}")

            # ===== iota-derived constant masks =====
            iota_i = singles.tile([128, 1], i32)
            nc.gpsimd.iota(iota_i, pattern=[[0, 1]], base=0, channel_multiplier=1)
            iota_f = singles.tile([128, 1], fp32)
            nc.vector.tensor_copy(iota_f, iota_i)

            colf_i = singles.tile([128, 128], i32)    # value j at col j
            nc.gpsimd.iota(colf_i, pattern=[[1, 128]], base=0, channel_multiplier=0)
            colf_f = singles.tile([128, 128], fp32)
            nc.vector.tensor_copy(colf_f, colf_i)

            ident = singles.tile([128, 128], fp32)    # identity matrix
            TS(ident, colf_f, iota_f, None, OP.is_equal)

            # mask32[p, b] = (p % 32 == b)
            pm32_i = singles.tile([128, 1], i32)
            TS(pm32_i, iota_i, 31, None, OP.bitwise_and)
            pm32_f = singles.tile([128, 1], fp32)
            nc.vector.tensor_copy(pm32_f, pm32_i)
            mask32 = singles.tile([128, 32], fp32)
            TS(mask32, colf_f[:, 0:32], pm32_f, None, OP.is_equal)

            # gqsel[p, q] = (p // 32 == q)
            pdiv32_i = singles.tile([128, 1], i32)
            TS(pdiv32_i, iota_i, 5, None, OP.arith_shift_right)
            pdiv32_f = singles.tile([128, 1], fp32)
            nc.vector.tensor_copy(pdiv32_f, pdiv32_i)
            gqsel = singles.tile([128, 4], fp32)
            TS(gqsel, colf_f[:, 0:4], pdiv32_f, None, OP.is_equal)

            # ===== weights =====
            # DMA exact fp32, then round to fp32r for the fp32r matmuls
            W1d = singles.tile([CD, H], fp32)
            dma(out=W1d, in_=io["W1"][:, :])
            W2d = [singles.tile([128, H], fp32, tag=f"w2d_{i}", name=f"w2d_{i}") for i in range(2)]
            W3d = [singles.tile([128, OUT3], fp32, tag=f"w3d_{i}", name=f"w3d_{i}") for i in range(2)]
            for i in range(2):
                dma(out=W2d[i], in_=io["W2"][i * 128:(i + 1) * 128, :])
                dma(out=W3d[i], in_=io["W3"][i * 128:(i + 1) * 128, :])
            W1s = singles.tile([CD, H], fp32r)
            nc.scalar.copy(W1s, W1d)
            W2s = [singles.tile([128, H], fp32r, tag=f"w2_{i}", name=f"w2_{i}") for i in range(2)]
            W3s = [singles.tile([128, OUT3], fp32r, tag=f"w3_{i}", name=f"w3_{i}") for i in range(2)]
            for i in range(2):
                nc.scalar.copy(W2s[i], W2d[i])
                nc.scalar.copy(W3s[i], W3d[i])
            b1t = singles.tile([128, 2], fp32)
            dma(out=b1t, in_=io["b1"].rearrange("(h p) -> p h", p=128))
            b2t = singles.tile([128, 2], fp32)
            dma(out=b2t, in_=io["b2"].rearrange("(h p) -> p h", p=128))
            b3t = singles.tile([OUT3, 1], fp32)
            dma(out=b3t, in_=io["b3"].rearrange("(o u) -> o u", u=1))

            # ===== MLP =====
            condT = singles.tile([CD, BL], fp32r)
            for t in range(T):
                csb = work.tile([128, CD], fp32, tag="cond", name="csb")
                dma(out=csb, in_=io["cond"][t * 128:(t + 1) * 128, :])
                ps = ps_tile(CD, 128)
                nc.tensor.transpose(ps, csb, ident)
                nc.scalar.copy(condT[:, t * 128:(t + 1) * 128], ps)

            h1 = [singles.tile([128, BL], fp32r, tag=f"h1_{i}", name=f"h1_{i}") for i in range(2)]
            for half in range(2):
                ps = ps_tile(128, BL)
                mmr(ps, W1s[:, half * 128:(half + 1) * 128], condT,
                    start=True, stop=True)
                nc.scalar.activation(h1[half], ps, AF.Relu, bias=b1t[:, half:half + 1])
            h2 = [singles.tile([128, BL], fp32r, tag=f"h2_{i}", name=f"h2_{i}") for i in range(2)]
            for half in range(2):
                ps = ps_tile(128, BL)
                for kc in range(2):
                    mmr(ps, W2s[kc][:, half * 128:(half + 1) * 128], h1[kc],
                        start=(kc == 0), stop=(kc == 1))
                nc.scalar.activation(h2[half], ps, AF.Relu, bias=b2t[:, half:half + 1])
            p_f = singles.tile([OUT3, BL], fp32)
            ps49 = ps_tile(OUT3, BL)
            for kc in range(2):
                mmr(ps49, W3s[kc], h2[kc], start=(kc == 0), stop=(kc == 1))
            nc.scalar.activation(p_f, ps49, AF.Identity, bias=b3t)

            pw = singles.tile([128, T, OUT3], fp32)   # p row-major
            for t in range(T):
                ps = ps_tile(128, OUT3)
                nc.tensor.transpose(ps, p_f[:, t * 128:(t + 1) * 128], ident[:OUT3, :OUT3])
                nc.scalar.copy(pw[:, t, :], ps)

            # ===== param pipeline =====
            un_w = pw[:, :, 0:K]
            un_h = pw[:, :, K:2 * K]
            un_d = pw[:, :, 2 * K:3 * K + 1]

            def softmax_w(un, mb, tag):
                mx = singles.tile([128, T], fp32, tag=f"mx{tag}", name=f"mx{tag}")
                nc.vector.tensor_reduce(mx, un, axis=AX.X, op=OP.max)
                nmx = singles.tile([128, T], fp32, tag=f"nmx{tag}", name=f"nmx{tag}")
                TS(nmx, mx, -1.0, None, OP.mult)
                ein = singles.tile([128, T, K], fp32, tag=f"ein{tag}", name=f"ein{tag}")
                for t in range(T):
                    TS(ein[:, t, :], un[:, t, :], nmx[:, t:t + 1], None, OP.add)
                ex = singles.tile([128, T, K], fp32, tag=f"ex{tag}", name=f"ex{tag}")
                nc.scalar.activation(ex, ein, AF.Exp)
                sm = singles.tile([128, T], fp32, tag=f"sm{tag}", name=f"sm{tag}")
                nc.vector.tensor_reduce(sm, ex, axis=AX.X, op=OP.add)
                rs = singles.tile([128, T], fp32, tag=f"rs{tag}", name=f"rs{tag}")
                nc.vector.reciprocal(rs, sm)
                wd = singles.tile([128, T, K], fp32, tag=f"wd{tag}", name=f"wd{tag}")
                for t in range(T):
                    TS(wd[:, t, :], ex[:, t, :], rs[:, t:t + 1], 2 * BOUND - K * mb,
                       OP.mult, OP.mult)
                TS(wd, wd, mb, None, OP.add)
                return wd

            widths = softmax_w(un_w, MBW, "w")
            heights = softmax_w(un_h, MBH, "h")

            zeros16 = singles.tile([128, K], fp32)
            nc.vector.memset(zeros16, 0.0)
            cumw = singles.tile([128, T, K + 1], fp32)
            cumh = singles.tile([128, T, K + 1], fp32)
            nc.vector.memset(cumw[:, :, 0:1], -BOUND)
            nc.vector.memset(cumh[:, :, 0:1], -BOUND)
            for t in range(T):
                nc.vector.tensor_tensor_scan(cumw[:, t, 1:], widths[:, t, :], zeros16,
                                             -BOUND, OP.add, OP.add)
                nc.vector.tensor_tensor_scan(cumh[:, t, 1:], heights[:, t, :], zeros16,
                                             -BOUND, OP.add, OP.add)

            # softplus(x) = max(x,0) + ln(1 + exp(-|x|))
            deriv = singles.tile([128, T, K + 1], fp32)
            absd = singles.tile([128, T, K + 1], fp32)
            nc.scalar.activation(absd, un_d, AF.Abs)
            end_ = singles.tile([128, T, K + 1], fp32)
            nc.scalar.activation(end_, absd, AF.Exp, scale=-1.0)
            l1p = singles.tile([128, T, K + 1], fp32)
            nc.scalar.activation(l1p, end_, AF.Ln, bias=1.0)
            rl = singles.tile([128, T, K + 1], fp32)
            TS(rl, un_d, 0.0, MD, OP.max, OP.add)
            TT(deriv, rl, l1p, OP.add)

            d0 = deriv[:, :, 0:K]
            d1 = deriv[:, :, 1:K + 1]
            y0 = cumh[:, :, 0:K]
            kx = cumw[:, :, 0:K]

            def tmp(tag):
                return singles.tile([128, T, K], fp32, tag=tag, name=tag)

            iw = tmp("iw"); nc.vector.reciprocal(iw, widths)
            delta = tmp("delta"); TT(delta, heights, iw, OP.mult)
            rdelta = tmp("rdelta"); nc.vector.reciprocal(rdelta, delta)
            s = tmp("s")
            for t in range(T):
                TS(s[:, t, :], rdelta[:, t, :], delta[:, t, 0:1], None, OP.mult)
            sig = tmp("sig"); TT(sig, d0, d1, OP.add)
            STT(sig, delta, -2.0, sig, OP.mult, OP.add)
            sdelta = tmp("sdelta"); TT(sdelta, s, delta, OP.mult)
            ssig = tmp("ssig"); TT(ssig, s, sig, OP.mult)
            sh = tmp("sh"); TT(sh, s, heights, OP.mult)
            shd0 = tmp("shd0"); TT(shd0, sh, d0, OP.mult)
            t1 = tmp("t1"); TT(t1, y0, ssig, OP.mult)
            Nc1 = tmp("Nc1"); TT(Nc1, t1, shd0, OP.add)
            u1 = tmp("u1"); TT(u1, delta, d0, OP.subtract)
            u2 = tmp("u2"); TT(u2, sh, u1, OP.mult)
            Nc2 = tmp("Nc2"); TT(Nc2, u2, t1, OP.subtract)
            sd2 = tmp("sd2"); TT(sd2, sdelta, sdelta, OP.mult)
            Cc1 = tmp("Cc1"); STT(Cc1, sd2, 2.0, u1, OP.mult, OP.mult)
            Cc2 = tmp("Cc2"); TT(Cc2, sd2, sig, OP.mult)
            iw2 = tmp("iw2"); TT(iw2, iw, iw, OP.mult)

            # final coefs, ci-minor: coefcat[:, t, k, ci]
            # ci: 0=aN 1=aD 2=aC 3=bN 4=bD 5=bC  (sq triple, lin triple)
            coefcat = singles.tile([128, T, K, 6], fp32)
            aN = coefcat[:, :, :, 0]; TT(aN, Nc2, iw2, OP.mult)
            aD = coefcat[:, :, :, 1]; STT(aD, ssig, -1.0, iw2, OP.mult, OP.mult)
            aC = coefcat[:, :, :, 2]; TT(aC, Cc2, iw2, OP.mult)
            bN = coefcat[:, :, :, 3]; TT(bN, Nc1, iw, OP.mult)
            bD = coefcat[:, :, :, 4]; TT(bD, ssig, iw, OP.mult)
            bC = coefcat[:, :, :, 5]; TT(bC, Cc1, iw, OP.mult)

            # per-row constants (poly order N, D, C)
            constcat = singles.tile([128, 3, T], fp32)
            constN = constcat[:, 0, :]
            TT(constN, y0[:, :, 0], sdelta[:, :, 0], OP.mult)
            constD = constcat[:, 1, :]
            nc.vector.tensor_copy(constD, sdelta[:, :, 0])
            constC = constcat[:, 2, :]
            TT(constC, sd2[:, :, 0], d0[:, :, 0], OP.mult)

            # fold sum_k b_k * kx_k into the constants (streams are raw t_k)
            for cst, b in ((constN, bN), (constD, bD), (constC, bC)):
                bx = tmp("bx"); TT(bx, b, kx, OP.mult)
                sbx = singles.tile([128, T], fp32, tag="sbx", name="sbx")
                nc.vector.tensor_reduce(sbx, bx, axis=AX.X, op=OP.add)
                TT(cst, cst, sbx, OP.subtract)

            # ===== CPK2[p=(32*po+b), t, Q] = const_po[32Q+b, t] via PE ====
            psCPK = pssetup.tile([96, T * QQ], fp32, tag="ps", name="psCPK")
            for t in range(T):
                cE4 = work.tile([128, 3, 32], fp32, tag="cE4", name="cE4")
                cin = constcat[:, :, t].unsqueeze(2).broadcast_to([128, 3, 32])
                m32 = mask32.unsqueeze(1).broadcast_to([128, 3, 32])
                TT(cE4, cin, m32, OP.mult)
                nc.tensor.matmul(psCPK[:, t * QQ:(t + 1) * QQ],
                                 cE4.rearrange("p a b -> p (a b)"), gqsel,
                                 start=True, stop=True)
            CPK2 = singles.tile([96, T, QQ], fp32)
            nc.scalar.copy(CPK2.rearrange("p a b -> p (a b)"), psCPK)

            # ===== stage cumw/coefcat to DRAM; gather into lane layout ====
            # lane layout: partition p = 32*m + b  (m = bin lane, b = row in
            # Q-block); bin k = 4*m + h
            cumwT = singles.tile([128, K + 1, T], fp32)   # k-major, t-minor
            nc.vector.tensor_copy(cumwT, cumw.rearrange("p t k -> p k t"))
            dramKX = dscr.tile([128, (K + 1) * T], fp32, name="dramKX")
            dma(out=dramKX, in_=cumwT.rearrange("p a b -> p (a b)"))
            dramCF = dscr.tile([128, T * K * 6], fp32, name="dramCF")
            dma(out=dramCF, in_=coefcat.rearrange("p a b c -> p (a b c)"))

            KX4 = singles.tile([128, QQ, HH, T], fp32)
            KX14 = singles.tile([128, QQ, HH, T], fp32)
            for Qb in range(QQ):
                # src addr(m, b, h, t) = (32Q+b)*68 + (4m+h)*4 + t
                base = dramKX.offset + 32 * Qb * ((K + 1) * T)
                src = bass.AP(tensor=dramKX.tensor, offset=base,
                              ap=[[4 * T, QQ], [(K + 1) * T, 32], [1, HH * T]])
                nc.scalar.dma_start(out=KX4[:, Qb, :, :], in_=src)
                src1 = bass.AP(tensor=dramKX.tensor, offset=base + T,
                               ap=[[4 * T, QQ], [(K + 1) * T, 32], [1, HH * T]])
                nc.scalar.dma_start(out=KX14[:, Qb, :, :], in_=src1)
            NEGKX4 = singles.tile([128, QQ, HH, T], fp32)
            TS(NEGKX4, KX4, -1.0, None, OP.mult)

            # coefP4[p=(32m+b), Q, t, h, (s,po)]
            coefP4 = singles.tile([128, QQ, T, HH, 6], fp32)
            for Qb in range(QQ):
                for t in range(T):
                    base = dramCF.offset + 32 * Qb * (T * K * 6) + t * (K * 6)
                    src = bass.AP(tensor=dramCF.tensor, offset=base,
                                  ap=[[4 * 6, QQ], [T * K * 6, 32], [1, HH * 6]])
                    (nc.scalar.dma_start if (Qb + t) % 2 else dma)(out=coefP4[:, Qb, t, :, :], in_=src)

        # ================= main loop ===================================
        with tc.tile_pool(name="psum_acc", bufs=2, space="PSUM") as psum_acc:
            for t in range(T):
                # stationaries for this t: LHS4[p, Q, s, h, po, b]
                # (s=0 -> sq coefs a, s=1 -> lin coefs b; col = 32*po + b)
                LHS4 = lhsp.tile([128, QQ, 2, HH, 3, 32], fp32r, tag="lhs",
                                 name=f"lhs{t}")
                for Qb in range(QQ):
                    cslice = coefP4[:, Qb, t, :, :]\
                        .rearrange("p h (s o) -> p s h o", s=2)\
                        .unsqueeze(4).broadcast_to([128, 2, HH, 3, 32])
                    m32b = mask32.unsqueeze(1).unsqueeze(1).unsqueeze(1)\
                        .broadcast_to([128, 2, HH, 3, 32])
                    TT(LHS4[:, Qb], cslice, m32b, OP.mult)

                for c in range(CH):
                    ACC = psum_acc.tile([96, QQ * 512], fp32, tag="ACC",
                                        name="ACC")
                    for Qb in range(QQ):
                        yrep = work.tile([128, 512], fp32, tag="yrep",
                                         name="yrep")
                        src = io["y"][t * 128 + 32 * Qb:t * 128 + 32 * Qb + 32,
                                      c * 512:(c + 1) * 512]
                        dma(out=yrep.rearrange("(m b) j -> m b j", b=32),
                            in_=src.unsqueeze(0).broadcast_to([4, 32, 512]))
                        slot = ACC[:, Qb * 512:(Qb + 1) * 512]
                        for h in range(HH):
                            tk = work.tile([128, 512], fp32r, tag="tk",
                                           name="tk")
                            TS(tk, yrep, KX4[:, Qb, h, t:t + 1],
                               KX14[:, Qb, h, t:t + 1], OP.max, OP.min)
                            usq = work.tile([128, 512], fp32r, tag="usq",
                                            name="usq")
                            nc.scalar.activation(usq, tk, AF.Square,
                                                 bias=NEGKX4[:, Qb, h, t:t + 1])
                            ll = LHS4[:, Qb, 1, h].rearrange("p a b -> p (a b)")
                            lq = LHS4[:, Qb, 0, h].rearrange("p a b -> p (a b)")
                            mmr(slot, ll, tk, start=(h == 0), stop=False)
                            mmr(slot, lq, usq, start=False, stop=(h == HH - 1))

                    # PSUM -> SBUF with per-row consts folded in
                    SACC = fin.tile([96, QQ, 512], fp32, tag="SACC", name="SACC")
                    for Qb in range(QQ):
                        if Qb % 2 == 0:
                            nc.scalar.activation(
                                SACC[:, Qb, :], ACC[:, Qb * 512:(Qb + 1) * 512],
                                AF.Identity, bias=CPK2[:, t, Qb:Qb + 1])
                        else:
                            TS(SACC[:, Qb, :], ACC[:, Qb * 512:(Qb + 1) * 512],
                               CPK2[:, t, Qb:Qb + 1], None, OP.add)

                    # un-interleave (po, b) rows -> batch rows: 12 SBUF->SBUF
                    polys = []
                    for pi in range(3):
                        dstt = fin.tile([128, 512], fp32, tag=f"poly{pi}",
                                        name=f"poly{pi}")
                        polys.append(dstt)
                    dmaq = [dma, nc.scalar.dma_start, nc.gpsimd.dma_start]
                    for pi in range(3):
                        for Qb in range(QQ):
                            dmaq[(pi + Qb) % 3](
                                out=polys[pi][32 * Qb:32 * Qb + 32, :],
                                in_=SACC[32 * pi:32 * pi + 32, Qb, :])
                    Np, Dp, Cp = polys

                    # finale
                    ysl = fin.tile([128, 512], fp32, tag="ysl", name="ysl")
                    dma(out=ysl,
                        in_=io["y"][t * 128:(t + 1) * 128, c * 512:(c + 1) * 512])
                    xsl = fin.tile([128, 512], fp32, tag="xsl", name="xsl")
                    nc.gpsimd.tensor_scalar(xsl, ysl, -BOUND, BOUND, OP.max, OP.min)
                    Cm = fin.tile([128, 512], fp32, tag="Cm", name="Cm")
                    nc.gpsimd.tensor_scalar(Cm, Cp, 1e-12, None, OP.max)
                    rD = fin.tile([128, 512], fp32, tag="rD", name="rD")
                    nc.vector.reciprocal(rD, Dp)
                    out0 = fin.tile([128, 512], fp32, tag="out0", name="out0")
                    nc.gpsimd.tensor_tensor(out0, Np, rD, OP.mult)
                    logD = fin.tile([128, 512], fp32, tag="logD", name="logD")
                    nc.scalar.activation(logD, Dp, AF.Ln)
                    logC = fin.tile([128, 512], fp32, tag="logC", name="logC")
                    nc.scalar.activation(logC, Cm, AF.Ln)
                    ld0 = fin.tile([128, 512], fp32, tag="ld0", name="ld0")
                    STT(ld0, logD, -2.0, logC, OP.mult, OP.add)
                    ee = fin.tile([128, 512], fp32, tag="ee", name="ee")
                    nc.gpsimd.tensor_tensor(ee, ysl, xsl, OP.subtract)
                    inz = fin.tile([128, 512], fp32, tag="inz", name="inz")
                    nc.gpsimd.tensor_scalar(inz, ee, 0.0, None, OP.is_equal)
                    outF = fin.tile([128, 512], fp32, tag="outF", name="outF")
                    nc.gpsimd.tensor_tensor(outF, out0, ee, OP.add)
                    ldF = fin.tile([128, 512], fp32, tag="ldF", name="ldF")
                    nc.gpsimd.tensor_tensor(ldF, ld0, inz, OP.mult)
                    dma(out=io["out"][t * 128:(t + 1) * 128,
                                      c * 512:(c + 1) * 512],
                        in_=outF)
                    nc.scalar.dma_start(
                        out=io["logdet"][t * 128:(t + 1) * 128,
                                         c * 512:(c + 1) * 512],
                        in_=ldF)


def kernel(cond, y, W1, b1, W2, b2, W3, b3):
    _ensure_path()
    from concourse.bass_utils import run_bass_kernel_spmd

    if "nc" not in _CACHE:
        _CACHE["nc"] = _build_nc()
    nc = _CACHE["nc"]

    cond = np.ascontiguousarray(cond, np.float32)
    y = np.ascontiguousarray(y, np.float32)
    shared = dict(W1=np.ascontiguousarray(W1, np.float32),
                  b1=np.ascontiguousarray(b1, np.float32),
                  W2=np.ascontiguousarray(W2, np.float32),
                  b2=np.ascontiguousarray(b2, np.float32),
                  W3=np.ascontiguousarray(W3, np.float32),
                  b3=np.ascontiguousarray(b3, np.float32))
    in_maps = []
    for i in range(NCORES):
        sl = slice(i * BL, (i + 1) * BL)
        in_maps.append(dict(cond=cond[sl], y=y[sl], **shared))
    res = run_bass_kernel_spmd(nc, in_maps, core_ids=list(range(NCORES)))
    out = np.concatenate([r["out"] for r in res.results], axis=0)
    ld = np.concatenate([r["logdet"] for r in res.results], axis=0)
    return out, ld


# revision 17
# speedup vs baseline: 2.0154x; 1.1994x over previous
"""Trainium2 Bass kernel for nn_ConditionalSpline1DFlow (K=16 RQS flow).

Data-parallel over 8 cores (B=4096 -> 512 rows/core). Per core:
  1. Conditioner MLP on TensorE (feature-major, fp32r matmuls).
  2. Spline params per row; rescale bin k's rational-quadratic by
     s_k = delta_0/delta_k so numerator N, denominator D and
     derivative-numerator C become globally CONTINUOUS piecewise
     quadratics in x.
  3. Evaluate N, D, C gather-free in the clipped-ramp basis
        P(x) = const + sum_k a_k*t_k^2' + b_k*t_k (consts folded),
        t_k = clip(x, x_k, x_{k+1})
     on TensorE with fp32r (1 cycle/row): partitions p = 32*m + b
     hold lane-bin m (bin k = 4m+h over 4 h-passes) for batch row b
     (32 rows/Q-block); x replicated 4x by DMA broadcast from DRAM.
     One [128, 96=(3 poly x 32 b)] stationary per (Q,s,h); all matmul
     dsts are base-partition-0 (fp32r ISA requirement).
  4. out = N/D + (y - clip(y)); logdet = (ln C - 2 ln D) * (y == clip(y)).
"""
import sys
import numpy as np

K = 16
BOUND = 5.0
MBW = 1e-3
MBH = 1e-3
MD = 1e-3
B_FULL, N = 4096, 1024
CD, H = 64, 256
OUT3 = 3 * K + 1
NCORES = 8
BL = B_FULL // NCORES   # 512 rows per core
T = BL // 128           # 4 partition tiles
CH = N // 512           # 2 free-dim chunks
QQ = 4                  # 32-row blocks per tile
HH = 4                  # h-passes (bins k = 4m+h)

_CACHE = {}


def _ensure_path():
    for p in ("/opt/trn_rl_repo",):
        if p not in sys.path:
            sys.path.insert(0, p)


def _build_nc():
    _ensure_path()
    import concourse.bacc as bacc
    import concourse.tile as tile
    from concourse import mybir

    fp32 = mybir.dt.float32
    nc = bacc.Bacc("TRN2", target_bir_lowering=False, debug=False)

    io = dict(
        cond=nc.dram_tensor("cond", [BL, CD], fp32, kind="ExternalInput"),
        y=nc.dram_tensor("y", [BL, N], fp32, kind="ExternalInput"),
        W1=nc.dram_tensor("W1", [CD, H], fp32, kind="ExternalInput"),
        b1=nc.dram_tensor("b1", [H], fp32, kind="ExternalInput"),
        W2=nc.dram_tensor("W2", [H, H], fp32, kind="ExternalInput"),
        b2=nc.dram_tensor("b2", [H], fp32, kind="ExternalInput"),
        W3=nc.dram_tensor("W3", [H, OUT3], fp32, kind="ExternalInput"),
        b3=nc.dram_tensor("b3", [OUT3], fp32, kind="ExternalInput"),
        out=nc.dram_tensor("out", [BL, N], fp32, kind="ExternalOutput"),
        logdet=nc.dram_tensor("logdet", [BL, N], fp32, kind="ExternalOutput"),
    )
    with tile.TileContext(nc) as tc:
        _emit(nc, tc, io)
    nc.compile()
    return nc


def _emit(nc, tc, io):
    from contextlib import ExitStack
    import concourse.bass as bass
    from concourse import mybir

    fp32 = mybir.dt.float32
    fp32r = mybir.dt.float32r
    i32 = mybir.dt.int32
    AF = mybir.ActivationFunctionType
    OP = mybir.AluOpType
    AX = mybir.AxisListType

    TT = nc.vector.tensor_tensor
    TS = nc.vector.tensor_scalar
    STT = nc.vector.scalar_tensor_tensor

    def mmr(out, lhsT, rhs, **kw):
        # fp32r (TF32-like): 1 cycle/row on PE when moving free dim >= 256
        # (vs 4 for fp32). Both operands produced as fp32r; dst base
        # partition must be 0 (ISA).
        nc.tensor.matmul(out, lhsT, rhs, **kw)

    ctx = ExitStack()
    with ctx:
        singles = ctx.enter_context(tc.tile_pool(name="singles", bufs=1))
        work = ctx.enter_context(tc.tile_pool(name="work", bufs=3))
        lhsp = ctx.enter_context(tc.tile_pool(name="lhsp", bufs=2))
        fin = ctx.enter_context(tc.tile_pool(name="fin", bufs=2))
        dscr = ctx.enter_context(tc.tile_pool(name="dscr", bufs=1, space="DRAM"))

        dma = nc.sync.dma_start

        # ================= setup phase (uses its own PSUM pool) ========
        with tc.tile_pool(name="ps_setup", bufs=2, space="PSUM") as pssetup:
            cnt = [0]

            def ps_tile(p, f):
                cnt[0] += 1
                return pssetup.tile([p, f], fp32, tag="ps", name=f"ps{cnt[0]}")

            # ===== iota-derived constant masks =====
            iota_i = singles.tile([128, 1], i32)
            nc.gpsimd.iota(iota_i, pattern=[[0, 1]], base=0, channel_multiplier=1)
            iota_f = singles.tile([128, 1], fp32)
            nc.vector.tensor_copy(iota_f, iota_i)

            colf_i = singles.tile([128, 128], i32)    # value j at col j
            nc.gpsimd.iota(colf_i, pattern=[[1, 128]], base=0, channel_multiplier=0)
            colf_f = singles.tile([128, 128], fp32)
            nc.vector.tensor_copy(colf_f, colf_i)

            ident = singles.tile([128, 128], fp32)    # identity matrix
            TS(ident, colf_f, iota_f, None, OP.is_equal)

            # mask32[p, b] = (p % 32 == b)
            pm32_i = singles.tile([128, 1], i32)
            TS(pm32_i, iota_i, 31, None, OP.bitwise_and)
            pm32_f = singles.tile([128, 1], fp32)
            nc.vector.tensor_copy(pm32_f, pm32_i)
            mask32 = singles.tile([128, 32], fp32)
            TS(mask32, colf_f[:, 0:32], pm32_f, None, OP.is_equal)

            # gqsel[p, q] = (p // 32 == q)
            pdiv32_i = singles.tile([128, 1], i32)
            TS(pdiv32_i, iota_i, 5, None, OP.arith_shift_right)
            pdiv32_f = singles.tile([128, 1], fp32)
            nc.vector.tensor_copy(pdiv32_f, pdiv32_i)
            gqsel = singles.tile([128, 4], fp32)
            TS(gqsel, colf_f[:, 0:4], pdiv32_f, None, OP.is_equal)

            # ===== weights =====
            # DMA exact fp32, then round to fp32r for the fp32r matmuls
            W1d = singles.tile([CD, H], fp32)
            dma(out=W1d, in_=io["W1"][:, :])
            W2d = [singles.tile([128, H], fp32, tag=f"w2d_{i}", name=f"w2d_{i}") for i in range(2)]
            W3d = [singles.tile([128, OUT3], fp32, tag=f"w3d_{i}", name=f"w3d_{i}") for i in range(2)]
            for i in range(2):
                dma(out=W2d[i], in_=io["W2"][i * 128:(i + 1) * 128, :])
                dma(out=W3d[i], in_=io["W3"][i * 128:(i + 1) * 128, :])
            W1s = singles.tile([CD, H], fp32r)
            nc.scalar.copy(W1s, W1d)
            W2s = [singles.tile([128, H], fp32r, tag=f"w2_{i}", name=f"w2_{i}") for i in range(2)]
            W3s = [singles.tile([128, OUT3], fp32r, tag=f"w3_{i}", name=f"w3_{i}") for i in range(2)]
            for i in range(2):
                nc.scalar.copy(W2s[i], W2d[i])
                nc.scalar.copy(W3s[i], W3d[i])
            b1t = singles.tile([128, 2], fp32)
            dma(out=b1t, in_=io["b1"].rearrange("(h p) -> p h", p=128))
            b2t = singles.tile([128, 2], fp32)
            dma(out=b2t, in_=io["b2"].rearrange("(h p) -> p h", p=128))
            b3t = singles.tile([OUT3, 1], fp32)
            dma(out=b3t, in_=io["b3"].rearrange("(o u) -> o u", u=1))

            # ===== MLP =====
            condT = singles.tile([CD, BL], fp32r)
            for t in range(T):
                csb = work.tile([128, CD], fp32, tag="cond", name="csb")
                dma(out=csb, in_=io["cond"][t * 128:(t + 1) * 128, :])
                ps = ps_tile(CD, 128)
                nc.tensor.transpose(ps, csb, ident)
                nc.scalar.copy(condT[:, t * 128:(t + 1) * 128], ps)

            h1 = [singles.tile([128, BL], fp32r, tag=f"h1_{i}", name=f"h1_{i}") for i in range(2)]
            for half in range(2):
                ps = ps_tile(128, BL)
                mmr(ps, W1s[:, half * 128:(half + 1) * 128], condT,
                    start=True, stop=True)
                nc.scalar.activation(h1[half], ps, AF.Relu, bias=b1t[:, half:half + 1])
            h2 = [singles.tile([128, BL], fp32r, tag=f"h2_{i}", name=f"h2_{i}") for i in range(2)]
            for half in range(2):
                ps = ps_tile(128, BL)
                for kc in range(2):
                    mmr(ps, W2s[kc][:, half * 128:(half + 1) * 128], h1[kc],
                        start=(kc == 0), stop=(kc == 1))
                nc.scalar.activation(h2[half], ps, AF.Relu, bias=b2t[:, half:half + 1])
            p_f = singles.tile([OUT3, BL], fp32)
            ps49 = ps_tile(OUT3, BL)
            for kc in range(2):
                mmr(ps49, W3s[kc], h2[kc], start=(kc == 0), stop=(kc == 1))
            nc.scalar.activation(p_f, ps49, AF.Identity, bias=b3t)

            pw = singles.tile([128, T, OUT3], fp32)   # p row-major
            for t in range(T):
                ps = ps_tile(128, OUT3)
                nc.tensor.transpose(ps, p_f[:, t * 128:(t + 1) * 128], ident[:OUT3, :OUT3])
                nc.scalar.copy(pw[:, t, :], ps)

            # ===== param pipeline =====
            un_w = pw[:, :, 0:K]
            un_h = pw[:, :, K:2 * K]
            un_d = pw[:, :, 2 * K:3 * K + 1]

            def softmax_w(un, mb, tag):
                mx = singles.tile([128, T], fp32, tag=f"mx{tag}", name=f"mx{tag}")
                nc.vector.tensor_reduce(mx, un, axis=AX.X, op=OP.max)
                nmx = singles.tile([128, T], fp32, tag=f"nmx{tag}", name=f"nmx{tag}")
                TS(nmx, mx, -1.0, None, OP.mult)
                ein = singles.tile([128, T, K], fp32, tag=f"ein{tag}", name=f"ein{tag}")
                for t in range(T):
                    TS(ein[:, t, :], un[:, t, :], nmx[:, t:t + 1], None, OP.add)
                ex = singles.tile([128, T, K], fp32, tag=f"ex{tag}", name=f"ex{tag}")
                nc.scalar.activation(ex, ein, AF.Exp)
                sm = singles.tile([128, T], fp32, tag=f"sm{tag}", name=f"sm{tag}")
                nc.vector.tensor_reduce(sm, ex, axis=AX.X, op=OP.add)
                rs = singles.tile([128, T], fp32, tag=f"rs{tag}", name=f"rs{tag}")
                nc.vector.reciprocal(rs, sm)
                wd = singles.tile([128, T, K], fp32, tag=f"wd{tag}", name=f"wd{tag}")
                for t in range(T):
                    TS(wd[:, t, :], ex[:, t, :], rs[:, t:t + 1], 2 * BOUND - K * mb,
                       OP.mult, OP.mult)
                TS(wd, wd, mb, None, OP.add)
                return wd

            widths = softmax_w(un_w, MBW, "w")
            heights = softmax_w(un_h, MBH, "h")

            zeros16 = singles.tile([128, K], fp32)
            nc.vector.memset(zeros16, 0.0)
            cumw = singles.tile([128, T, K + 1], fp32)
            cumh = singles.tile([128, T, K + 1], fp32)
            nc.vector.memset(cumw[:, :, 0:1], -BOUND)
            nc.vector.memset(cumh[:, :, 0:1], -BOUND)
            for t in range(T):
                nc.vector.tensor_tensor_scan(cumw[:, t, 1:], widths[:, t, :], zeros16,
                                             -BOUND, OP.add, OP.add)
                nc.vector.tensor_tensor_scan(cumh[:, t, 1:], heights[:, t, :], zeros16,
                                             -BOUND, OP.add, OP.add)

            # softplus(x) = max(x,0) + ln(1 + exp(-|x|))
            deriv = singles.tile([128, T, K + 1], fp32)
            absd = singles.tile([128, T, K + 1], fp32)
            nc.scalar.activation(absd, un_d, AF.Abs)
            end_ = singles.tile([128, T, K + 1], fp32)
            nc.scalar.activation(end_, absd, AF.Exp, scale=-1.0)
            l1p = singles.tile([128, T, K + 1], fp32)
            nc.scalar.activation(l1p, end_, AF.Ln, bias=1.0)
            rl = singles.tile([128, T, K + 1], fp32)
            TS(rl, un_d, 0.0, MD, OP.max, OP.add)
            TT(deriv, rl, l1p, OP.add)

            d0 = deriv[:, :, 0:K]
            d1 = deriv[:, :, 1:K + 1]
            y0 = cumh[:, :, 0:K]
            kx = cumw[:, :, 0:K]

            def tmp(tag):
                return singles.tile([128, T, K], fp32, tag=tag, name=tag)

            iw = tmp("iw"); nc.vector.reciprocal(iw, widths)
            delta = tmp("delta"); TT(delta, heights, iw, OP.mult)
            rdelta = tmp("rdelta"); nc.vector.reciprocal(rdelta, delta)
            s = tmp("s")
            for t in range(T):
                TS(s[:, t, :], rdelta[:, t, :], delta[:, t, 0:1], None, OP.mult)
            sig = tmp("sig"); TT(sig, d0, d1, OP.add)
            STT(sig, delta, -2.0, sig, OP.mult, OP.add)
            sdelta = tmp("sdelta"); TT(sdelta, s, delta, OP.mult)
            ssig = tmp("ssig"); TT(ssig, s, sig, OP.mult)
            sh = tmp("sh"); TT(sh, s, heights, OP.mult)
            shd0 = tmp("shd0"); TT(shd0, sh, d0, OP.mult)
            t1 = tmp("t1"); TT(t1, y0, ssig, OP.mult)
            Nc1 = tmp("Nc1"); TT(Nc1, t1, shd0, OP.add)
            u1 = tmp("u1"); TT(u1, delta, d0, OP.subtract)
            u2 = tmp("u2"); TT(u2, sh, u1, OP.mult)
            Nc2 = tmp("Nc2"); TT(Nc2, u2, t1, OP.subtract)
            sd2 = tmp("sd2"); TT(sd2, sdelta, sdelta, OP.mult)
            Cc1 = tmp("Cc1"); STT(Cc1, sd2, 2.0, u1, OP.mult, OP.mult)
            Cc2 = tmp("Cc2"); TT(Cc2, sd2, sig, OP.mult)
            iw2 = tmp("iw2"); TT(iw2, iw, iw, OP.mult)

            # final coefs, ci-minor: coefcat[:, t, k, ci]
            # ci: 0=aN 1=aD 2=aC 3=bN 4=bD 5=bC  (sq triple, lin triple)
            coefcat = singles.tile([128, T, K, 6], fp32)
            aN = coefcat[:, :, :, 0]; TT(aN, Nc2, iw2, OP.mult)
            aD = coefcat[:, :, :, 1]; STT(aD, ssig, -1.0, iw2, OP.mult, OP.mult)
            aC = coefcat[:, :, :, 2]; TT(aC, Cc2, iw2, OP.mult)
            bN = coefcat[:, :, :, 3]; TT(bN, Nc1, iw, OP.mult)
            bD = coefcat[:, :, :, 4]; TT(bD, ssig, iw, OP.mult)
            bC = coefcat[:, :, :, 5]; TT(bC, Cc1, iw, OP.mult)

            # per-row constants (poly order N, D, C)
            constcat = singles.tile([128, 3, T], fp32)
            constN = constcat[:, 0, :]
            TT(constN, y0[:, :, 0], sdelta[:, :, 0], OP.mult)
            constD = constcat[:, 1, :]
            nc.vector.tensor_copy(constD, sdelta[:, :, 0])
            constC = constcat[:, 2, :]
            TT(constC, sd2[:, :, 0], d0[:, :, 0], OP.mult)

            # fold sum_k b_k * kx_k into the constants (streams are raw t_k)
            for cst, b in ((constN, bN), (constD, bD), (constC, bC)):
                bx = tmp("bx"); TT(bx, b, kx, OP.mult)
                sbx = singles.tile([128, T], fp32, tag="sbx", name="sbx")
                nc.vector.tensor_reduce(sbx, bx, axis=AX.X, op=OP.add)
                TT(cst, cst, sbx, OP.subtract)

            # ===== CPK2[p=(32*po+b), t, Q] = const_po[32Q+b, t] via PE ====
            psCPK = pssetup.tile([96, T * QQ], fp32, tag="ps", name="psCPK")
            for t in range(T):
                cE4 = work.tile([128, 3, 32], fp32, tag="cE4", name="cE4")
                cin = constcat[:, :, t].unsqueeze(2).broadcast_to([128, 3, 32])
                m32 = mask32.unsqueeze(1).broadcast_to([128, 3, 32])
                TT(cE4, cin, m32, OP.mult)
                nc.tensor.matmul(psCPK[:, t * QQ:(t + 1) * QQ],
                                 cE4.rearrange("p a b -> p (a b)"), gqsel,
                                 start=True, stop=True)
            CPK2 = singles.tile([96, T, QQ], fp32)
            nc.scalar.copy(CPK2.rearrange("p a b -> p (a b)"), psCPK)

            # ===== stage cumw/coefcat to DRAM; gather into lane layout ====
            # lane layout: partition p = 32*m + b  (m = bin lane, b = row in
            # Q-block); bin k = 4*m + h
            cumwT = singles.tile([128, K + 1, T], fp32)   # k-major, t-minor
            nc.vector.tensor_copy(cumwT, cumw.rearrange("p t k -> p k t"))
            dramKX = dscr.tile([128, (K + 1) * T], fp32, name="dramKX")
            dma(out=dramKX, in_=cumwT.rearrange("p a b -> p (a b)"))
            dramCF = dscr.tile([128, T * K * 6], fp32, name="dramCF")
            dma(out=dramCF, in_=coefcat.rearrange("p a b c -> p (a b c)"))

            KX4 = singles.tile([128, QQ, HH, T], fp32)
            KX14 = singles.tile([128, QQ, HH, T], fp32)
            for Qb in range(QQ):
                # src addr(m, b, h, t) = (32Q+b)*68 + (4m+h)*4 + t
                base = dramKX.offset + 32 * Qb * ((K + 1) * T)
                src = bass.AP(tensor=dramKX.tensor, offset=base,
                              ap=[[4 * T, QQ], [(K + 1) * T, 32], [1, HH * T]])
                nc.scalar.dma_start(out=KX4[:, Qb, :, :], in_=src)
                src1 = bass.AP(tensor=dramKX.tensor, offset=base + T,
                               ap=[[4 * T, QQ], [(K + 1) * T, 32], [1, HH * T]])
                nc.scalar.dma_start(out=KX14[:, Qb, :, :], in_=src1)
            NEGKX4 = singles.tile([128, QQ, HH, T], fp32)
            TS(NEGKX4, KX4, -1.0, None, OP.mult)

            # coefP4[p=(32m+b), Q, t, h, (s,po)]
            coefP4 = singles.tile([128, QQ, T, HH, 6], fp32)
            for t in range(T):
                for Qb in range(QQ):
                    base = dramCF.offset + 32 * Qb * (T * K * 6) + t * (K * 6)
                    src = bass.AP(tensor=dramCF.tensor, offset=base,
                                  ap=[[4 * 6, QQ], [T * K * 6, 32], [1, HH * 6]])
                    (nc.scalar.dma_start if (Qb + t) % 2 else dma)(out=coefP4[:, Qb, t, :, :], in_=src)

        # ================= main loop ===================================
        # Output-side partition scramble: poly/finale partition p holds batch
        # row 32*(p%4) + p//4 of the t-block, so each un-interleave is ONE
        # full-width SBUF->SBUF DMA (src (b,Q,j) order == dst p=(b,Q) order).
        # The y load and out/logdet stores use the same scrambled row AP.
        def yview(t, c):
            return io["y"][t * 128:(t + 1) * 128, c * 512:(c + 1) * 512]\
                .rearrange("(q b) j -> b q j", q=QQ)

        with tc.tile_pool(name="psum_acc", bufs=2, space="PSUM") as psum_acc:
            for t in range(T):
                # stationaries for this t: LHS4[p, Q, s, h, po, b]
                # (s=0 -> sq coefs a, s=1 -> lin coefs b; col = 32*po + b)
                LHS4 = lhsp.tile([128, QQ, 2, HH, 3, 32], fp32r, tag="lhs",
                                 name=f"lhs{t}")
                for Qb in range(QQ):
                    cslice = coefP4[:, Qb, t, :, :]\
                        .rearrange("p h (s o) -> p s h o", s=2)\
                        .unsqueeze(4).broadcast_to([128, 2, HH, 3, 32])
                    m32b = mask32.unsqueeze(1).unsqueeze(1).unsqueeze(1)\
                        .broadcast_to([128, 2, HH, 3, 32])
                    TT(LHS4[:, Qb], cslice, m32b, OP.mult)

                # both c-chunks' accumulators live together: matmuls issue
                # Q-major back-to-back to keep the PE p-state streak long
                ACCs = [psum_acc.tile([96, QQ * 512], fp32, tag="ACC",
                                      name=f"ACC{t}_{c}") for c in range(CH)]
                for Qb in range(QQ):
                    yrep = work.tile([128, N], fp32, tag="yrep", name="yrep")
                    src = io["y"][t * 128 + 32 * Qb:t * 128 + 32 * Qb + 32, :]
                    dma(out=yrep.rearrange("(m b) j -> m b j", b=32),
                        in_=src.unsqueeze(0).broadcast_to([4, 32, N]))
                    for h in range(HH):
                        tk = work.tile([128, N], fp32r, tag="tk", name="tk")
                        TS(tk, yrep, KX4[:, Qb, h, t:t + 1],
                           KX14[:, Qb, h, t:t + 1], OP.max, OP.min)
                        usq = work.tile([128, N], fp32r, tag="usq", name="usq")
                        nc.scalar.activation(usq, tk, AF.Square,
                                             bias=NEGKX4[:, Qb, h, t:t + 1])
                        ll = LHS4[:, Qb, 1, h].rearrange("p a b -> p (a b)")
                        lq = LHS4[:, Qb, 0, h].rearrange("p a b -> p (a b)")
                        for c in range(CH):
                            slot = ACCs[c][:, Qb * 512:(Qb + 1) * 512]
                            sl = slice(c * 512, (c + 1) * 512)
                            mmr(slot, ll, tk[:, sl], start=(h == 0), stop=False)
                            mmr(slot, lq, usq[:, sl], start=False,
                                stop=(h == HH - 1))

                for c in range(CH):
                    ACC = ACCs[c]
                    # PSUM -> SBUF with per-row consts folded in
                    SACC = fin.tile([96, QQ, 512], fp32, tag="SACC", name="SACC")
                    for Qb in range(QQ):
                        if Qb % 2 == 0:
                            nc.scalar.activation(
                                SACC[:, Qb, :], ACC[:, Qb * 512:(Qb + 1) * 512],
                                AF.Identity, bias=CPK2[:, t, Qb:Qb + 1])
                        else:
                            TS(SACC[:, Qb, :], ACC[:, Qb * 512:(Qb + 1) * 512],
                               CPK2[:, t, Qb:Qb + 1], None, OP.add)

                    # un-interleave: one full-width SBUF->SBUF DMA per poly
                    polys = []
                    for pi in range(3):
                        dstt = fin.tile([128, 512], fp32, tag=f"poly{pi}",
                                        name=f"poly{pi}")
                        dq = nc.scalar.dma_start if pi == 1 else dma
                        dq(out=dstt, in_=SACC[32 * pi:32 * pi + 32, :, :])
                        polys.append(dstt)
                    Np, Dp, Cp = polys

                    # finale (scrambled row order; all ops elementwise)
                    ysl = fin.tile([128, 512], fp32, tag="ysl", name="ysl")
                    nc.scalar.dma_start(out=ysl, in_=yview(t, c))
                    xsl = fin.tile([128, 512], fp32, tag="xsl", name="xsl")
                    nc.gpsimd.tensor_scalar(xsl, ysl, -BOUND, BOUND, OP.max,
                                            OP.min)
                    # xsl -> ee (in place), inz on DVE (2x mode)
                    nc.gpsimd.tensor_tensor(xsl, ysl, xsl, OP.subtract)
                    inz = fin.tile([128, 512], fp32, tag="inz", name="inz")
                    TS(inz, xsl, 0.0, None, OP.is_equal)
                    rD = fin.tile([128, 512], fp32, tag="rD", name="rD")
                    nc.vector.reciprocal(rD, Dp)
                    # Cp -> max(C,eps) -> C*rD*rD -> ln -> *inz  == logdet
                    nc.gpsimd.tensor_scalar(Cp, Cp, 1e-12, None, OP.max)
                    TT(Cp, Cp, rD, OP.mult)
                    TT(Cp, Cp, rD, OP.mult)
                    ld0 = fin.tile([128, 512], fp32, tag="ld0", name="ld0")
                    nc.scalar.activation(ld0, Cp, AF.Ln)
                    nc.gpsimd.tensor_tensor(ld0, ld0, inz, OP.mult)
                    # Np -> N*rD -> + (y - xc)  == out
                    out0 = fin.tile([128, 512], fp32, tag="out0", name="out0")
                    nc.gpsimd.tensor_tensor(out0, Np, rD, OP.mult)
                    nc.gpsimd.tensor_tensor(out0, out0, xsl, OP.add)
                    oview = io["out"][t * 128:(t + 1) * 128,
                                      c * 512:(c + 1) * 512]\
                        .rearrange("(q b) j -> b q j", q=QQ)
                    dma(out=oview, in_=out0.rearrange("(b q) j -> b q j", q=QQ))
                    lview = io["logdet"][t * 128:(t + 1) * 128,
                                         c * 512:(c + 1) * 512]\
                        .rearrange("(q b) j -> b q j", q=QQ)
                    nc.scalar.dma_start(
                        out=lview, in_=ld0.rearrange("(b q) j -> b q j", q=QQ))


def kernel(cond, y, W1, b1, W2, b2, W3, b3):
    _ensure_path()
    from concourse.bass_utils import run_bass_kernel_spmd

    if "nc" not in _CACHE:
        _CACHE["nc"] = _build_nc()
    nc = _CACHE["nc"]

    cond = np.ascontiguousarray(cond, np.float32)
    y = np.ascontiguousarray(y, np.float32)
    shared = dict(W1=np.ascontiguousarray(W1, np.float32),
                  b1=np.ascontiguousarray(b1, np.float32),
                  W2=np.ascontiguousarray(W2, np.float32),
                  b2=np.ascontiguousarray(b2, np.float32),
                  W3=np.ascontiguousarray(W3, np.float32),
                  b3=np.ascontiguousarray(b3, np.float32))
    in_maps = []
    for i in range(NCORES):
        sl = slice(i * BL, (i + 1) * BL)
        in_maps.append(dict(cond=cond[sl], y=y[sl], **shared))
    res = run_bass_kernel_spmd(nc, in_maps, core_ids=list(range(NCORES)))
    out = np.concatenate([r["out"] for r in res.results], axis=0)
    ld = np.concatenate([r["logdet"] for r in res.results], axis=0)
    return out, ld


# revision 19
# speedup vs baseline: 2.1641x; 1.0738x over previous
"""Trainium2 Bass kernel for nn_ConditionalSpline1DFlow (K=16 RQS flow).

Data-parallel over 8 cores (B=4096 -> 512 rows/core). Per core:
  1. Conditioner MLP on TensorE (feature-major, fp32r matmuls).
  2. Spline params per row; rescale bin k's rational-quadratic by
     s_k = delta_0/delta_k so numerator N, denominator D and
     derivative-numerator C become globally CONTINUOUS piecewise
     quadratics in x.
  3. Evaluate N, D, C gather-free in the clipped-ramp basis
        P(x) = const + sum_k a_k*t_k^2' + b_k*t_k (consts folded),
        t_k = clip(x, x_k, x_{k+1})
     on TensorE with fp32r (1 cycle/row): partitions p = 32*m + b
     hold lane-bin m (bin k = 4m+h over 4 h-passes) for batch row b
     (32 rows/Q-block); x replicated 4x by DMA broadcast from DRAM.
     One [128, 96=(3 poly x 32 b)] stationary per (Q,s,h); all matmul
     dsts are base-partition-0 (fp32r ISA requirement).
  4. out = N/D + (y - clip(y)); logdet = (ln C - 2 ln D) * (y == clip(y)).
"""
import sys
import numpy as np

K = 16
BOUND = 5.0
MBW = 1e-3
MBH = 1e-3
MD = 1e-3
B_FULL, N = 4096, 1024
CD, H = 64, 256
OUT3 = 3 * K + 1
NCORES = 8
BL = B_FULL // NCORES   # 512 rows per core
T = BL // 128           # 4 partition tiles
CH = N // 512           # 2 free-dim chunks
QQ = 4                  # 32-row blocks per tile
HH = 4                  # h-passes (bins k = 4m+h)

_CACHE = {}


def _ensure_path():
    for p in ("/opt/trn_rl_repo",):
        if p not in sys.path:
            sys.path.insert(0, p)


def _build_nc():
    _ensure_path()
    import concourse.bacc as bacc
    import concourse.tile as tile
    from concourse import mybir

    fp32 = mybir.dt.float32
    nc = bacc.Bacc("TRN2", target_bir_lowering=False, debug=False)

    io = dict(
        cond=nc.dram_tensor("cond", [BL, CD], fp32, kind="ExternalInput"),
        y=nc.dram_tensor("y", [BL, N], fp32, kind="ExternalInput"),
        W1=nc.dram_tensor("W1", [CD, H], fp32, kind="ExternalInput"),
        b1=nc.dram_tensor("b1", [H], fp32, kind="ExternalInput"),
        W2=nc.dram_tensor("W2", [H, H], fp32, kind="ExternalInput"),
        b2=nc.dram_tensor("b2", [H], fp32, kind="ExternalInput"),
        W3=nc.dram_tensor("W3", [H, OUT3], fp32, kind="ExternalInput"),
        b3=nc.dram_tensor("b3", [OUT3], fp32, kind="ExternalInput"),
        out=nc.dram_tensor("out", [BL, N], fp32, kind="ExternalOutput"),
        logdet=nc.dram_tensor("logdet", [BL, N], fp32, kind="ExternalOutput"),
    )
    with tile.TileContext(nc) as tc:
        _emit(nc, tc, io)
    nc.compile()
    return nc


def _emit(nc, tc, io):
    from contextlib import ExitStack
    import concourse.bass as bass
    from concourse import mybir

    fp32 = mybir.dt.float32
    fp32r = mybir.dt.float32r
    i32 = mybir.dt.int32
    AF = mybir.ActivationFunctionType
    OP = mybir.AluOpType
    AX = mybir.AxisListType

    TT = nc.vector.tensor_tensor
    TS = nc.vector.tensor_scalar
    STT = nc.vector.scalar_tensor_tensor

    def mmr(out, lhsT, rhs, **kw):
        # fp32r (TF32-like): 1 cycle/row on PE when moving free dim >= 256
        # (vs 4 for fp32). Both operands produced as fp32r; dst base
        # partition must be 0 (ISA).
        nc.tensor.matmul(out, lhsT, rhs, **kw)

    ctx = ExitStack()
    with ctx:
        singles = ctx.enter_context(tc.tile_pool(name="singles", bufs=1))
        work = ctx.enter_context(tc.tile_pool(name="work", bufs=3))
        lhsp = ctx.enter_context(tc.tile_pool(name="lhsp", bufs=2))
        fin = ctx.enter_context(tc.tile_pool(name="fin", bufs=2))
        dscr = ctx.enter_context(tc.tile_pool(name="dscr", bufs=1, space="DRAM"))

        dma = nc.sync.dma_start

        # ================= setup phase (uses its own PSUM pool) ========
        with tc.tile_pool(name="ps_setup", bufs=2, space="PSUM") as pssetup:
            cnt = [0]

            def ps_tile(p, f):
                cnt[0] += 1
                return pssetup.tile([p, f], fp32, tag="ps", name=f"ps{cnt[0]}")

            # ===== iota-derived constant masks =====
            iota_i = singles.tile([128, 1], i32)
            nc.gpsimd.iota(iota_i, pattern=[[0, 1]], base=0, channel_multiplier=1)
            iota_f = singles.tile([128, 1], fp32)
            nc.vector.tensor_copy(iota_f, iota_i)

            colf_i = singles.tile([128, 128], i32)    # value j at col j
            nc.gpsimd.iota(colf_i, pattern=[[1, 128]], base=0, channel_multiplier=0)
            colf_f = singles.tile([128, 128], fp32)
            nc.vector.tensor_copy(colf_f, colf_i)

            ident = singles.tile([128, 128], fp32)    # identity matrix
            TS(ident, colf_f, iota_f, None, OP.is_equal)

            # mask32[p, b] = (p % 32 == b)
            pm32_i = singles.tile([128, 1], i32)
            TS(pm32_i, iota_i, 31, None, OP.bitwise_and)
            pm32_f = singles.tile([128, 1], fp32)
            nc.vector.tensor_copy(pm32_f, pm32_i)
            mask32 = singles.tile([128, 32], fp32)
            TS(mask32, colf_f[:, 0:32], pm32_f, None, OP.is_equal)

            # gqsel[p, q] = (p // 32 == q)
            pdiv32_i = singles.tile([128, 1], i32)
            TS(pdiv32_i, iota_i, 5, None, OP.arith_shift_right)
            pdiv32_f = singles.tile([128, 1], fp32)
            nc.vector.tensor_copy(pdiv32_f, pdiv32_i)
            gqsel = singles.tile([128, 4], fp32)
            TS(gqsel, colf_f[:, 0:4], pdiv32_f, None, OP.is_equal)

            # ===== weights =====
            # DMA exact fp32, then round to fp32r for the fp32r matmuls
            W1d = singles.tile([CD, H], fp32)
            dma(out=W1d, in_=io["W1"][:, :])
            W2dt = singles.tile([128, 2, H], fp32)
            dma(out=W2dt, in_=io["W2"].rearrange("(i p) h -> p i h", p=128))
            W3dt = singles.tile([128, 2, OUT3], fp32)
            dma(out=W3dt, in_=io["W3"].rearrange("(i p) o -> p i o", p=128))
            W2d = [W2dt[:, i, :] for i in range(2)]
            W3d = [W3dt[:, i, :] for i in range(2)]
            W1s = singles.tile([CD, H], fp32r)
            nc.scalar.copy(W1s, W1d)
            W2s = [singles.tile([128, H], fp32r, tag=f"w2_{i}", name=f"w2_{i}") for i in range(2)]
            W3s = [singles.tile([128, OUT3], fp32r, tag=f"w3_{i}", name=f"w3_{i}") for i in range(2)]
            for i in range(2):
                nc.scalar.copy(W2s[i], W2d[i])
                nc.scalar.copy(W3s[i], W3d[i])
            b1t = singles.tile([128, 2], fp32)
            dma(out=b1t, in_=io["b1"].rearrange("(h p) -> p h", p=128))
            b2t = singles.tile([128, 2], fp32)
            dma(out=b2t, in_=io["b2"].rearrange("(h p) -> p h", p=128))
            b3t = singles.tile([OUT3, 1], fp32)
            dma(out=b3t, in_=io["b3"].rearrange("(o u) -> o u", u=1))

            # ===== MLP =====
            condT = singles.tile([CD, BL], fp32r)
            csb = singles.tile([128, T, CD], fp32)
            dma(out=csb, in_=io["cond"].rearrange("(t p) d -> p t d", p=128))
            for t in range(T):
                ps = ps_tile(CD, 128)
                nc.tensor.transpose(ps, csb[:, t, :], ident)
                nc.scalar.copy(condT[:, t * 128:(t + 1) * 128], ps)

            h1 = [singles.tile([128, BL], fp32r, tag=f"h1_{i}", name=f"h1_{i}") for i in range(2)]
            for half in range(2):
                ps = ps_tile(128, BL)
                mmr(ps, W1s[:, half * 128:(half + 1) * 128], condT,
                    start=True, stop=True)
                nc.scalar.activation(h1[half], ps, AF.Relu, bias=b1t[:, half:half + 1])
            h2 = [singles.tile([128, BL], fp32r, tag=f"h2_{i}", name=f"h2_{i}") for i in range(2)]
            for half in range(2):
                ps = ps_tile(128, BL)
                for kc in range(2):
                    mmr(ps, W2s[kc][:, half * 128:(half + 1) * 128], h1[kc],
                        start=(kc == 0), stop=(kc == 1))
                nc.scalar.activation(h2[half], ps, AF.Relu, bias=b2t[:, half:half + 1])
            p_f = singles.tile([OUT3, BL], fp32)
            ps49 = ps_tile(OUT3, BL)
            for kc in range(2):
                mmr(ps49, W3s[kc], h2[kc], start=(kc == 0), stop=(kc == 1))
            nc.scalar.activation(p_f, ps49, AF.Identity, bias=b3t)

            pw = singles.tile([128, T, OUT3], fp32)   # p row-major
            for t in range(T):
                ps = ps_tile(128, OUT3)
                nc.tensor.transpose(ps, p_f[:, t * 128:(t + 1) * 128], ident[:OUT3, :OUT3])
                nc.scalar.copy(pw[:, t, :], ps)

            # ===== param pipeline =====
            un_w = pw[:, :, 0:K]
            un_h = pw[:, :, K:2 * K]
            un_d = pw[:, :, 2 * K:3 * K + 1]

            def softmax_w(un, mb, tag):
                mx = singles.tile([128, T], fp32, tag=f"mx{tag}", name=f"mx{tag}")
                nc.vector.tensor_reduce(mx, un, axis=AX.X, op=OP.max)
                nmx = singles.tile([128, T], fp32, tag=f"nmx{tag}", name=f"nmx{tag}")
                TS(nmx, mx, -1.0, None, OP.mult)
                ein = singles.tile([128, T, K], fp32, tag=f"ein{tag}", name=f"ein{tag}")
                for t in range(T):
                    TS(ein[:, t, :], un[:, t, :], nmx[:, t:t + 1], None, OP.add)
                ex = singles.tile([128, T, K], fp32, tag=f"ex{tag}", name=f"ex{tag}")
                nc.scalar.activation(ex, ein, AF.Exp)
                sm = singles.tile([128, T], fp32, tag=f"sm{tag}", name=f"sm{tag}")
                nc.vector.tensor_reduce(sm, ex, axis=AX.X, op=OP.add)
                rs = singles.tile([128, T], fp32, tag=f"rs{tag}", name=f"rs{tag}")
                nc.vector.reciprocal(rs, sm)
                wd = singles.tile([128, T, K], fp32, tag=f"wd{tag}", name=f"wd{tag}")
                for t in range(T):
                    TS(wd[:, t, :], ex[:, t, :], rs[:, t:t + 1], 2 * BOUND - K * mb,
                       OP.mult, OP.mult)
                TS(wd, wd, mb, None, OP.add)
                return wd

            widths = softmax_w(un_w, MBW, "w")
            heights = softmax_w(un_h, MBH, "h")

            zeros16 = singles.tile([128, K], fp32)
            nc.vector.memset(zeros16, 0.0)
            cumw = singles.tile([128, T, K + 1], fp32)
            cumh = singles.tile([128, T, K + 1], fp32)
            nc.vector.memset(cumw[:, :, 0:1], -BOUND)
            nc.vector.memset(cumh[:, :, 0:1], -BOUND)
            for t in range(T):
                nc.vector.tensor_tensor_scan(cumw[:, t, 1:], widths[:, t, :], zeros16,
                                             -BOUND, OP.add, OP.add)
                nc.vector.tensor_tensor_scan(cumh[:, t, 1:], heights[:, t, :], zeros16,
                                             -BOUND, OP.add, OP.add)

            # ===== stage cumw to DRAM; gather clip bounds into lane ======
            # layout (early: only depends on cumw): partition p = 32*m + b
            # (m = bin lane, b = row in Q-block); bin k = 4*m + h
            cumwT = singles.tile([128, K + 1, T], fp32)   # k-major, t-minor
            nc.vector.tensor_copy(cumwT, cumw.rearrange("p t k -> p k t"))
            dramKX = dscr.tile([128, (K + 1) * T], fp32, name="dramKX")
            dma(out=dramKX, in_=cumwT.rearrange("p a b -> p (a b)"))
            KX4 = singles.tile([128, QQ, HH, T], fp32)
            KX14 = singles.tile([128, QQ, HH, T], fp32)
            for Qb in range(QQ):
                # src addr(m, b, h, t) = (32Q+b)*68 + (4m+h)*4 + t
                base = dramKX.offset + 32 * Qb * ((K + 1) * T)
                src = bass.AP(tensor=dramKX.tensor, offset=base,
                              ap=[[4 * T, QQ], [(K + 1) * T, 32], [1, HH * T]])
                nc.scalar.dma_start(out=KX4[:, Qb, :, :], in_=src)
                src1 = bass.AP(tensor=dramKX.tensor, offset=base + T,
                               ap=[[4 * T, QQ], [(K + 1) * T, 32], [1, HH * T]])
                nc.scalar.dma_start(out=KX14[:, Qb, :, :], in_=src1)
            NEGKX4 = singles.tile([128, QQ, HH, T], fp32)
            TS(NEGKX4, KX4, -1.0, None, OP.mult)

            # softplus(x) = max(x,0) + ln(1 + exp(-|x|))
            deriv = singles.tile([128, T, K + 1], fp32)
            absd = singles.tile([128, T, K + 1], fp32)
            nc.scalar.activation(absd, un_d, AF.Abs)
            end_ = singles.tile([128, T, K + 1], fp32)
            nc.scalar.activation(end_, absd, AF.Exp, scale=-1.0)
            l1p = singles.tile([128, T, K + 1], fp32)
            nc.scalar.activation(l1p, end_, AF.Ln, bias=1.0)
            rl = singles.tile([128, T, K + 1], fp32)
            TS(rl, un_d, 0.0, MD, OP.max, OP.add)
            TT(deriv, rl, l1p, OP.add)

            d0 = deriv[:, :, 0:K]
            d1 = deriv[:, :, 1:K + 1]
            y0 = cumh[:, :, 0:K]
            kx = cumw[:, :, 0:K]

            def tmp(tag):
                return singles.tile([128, T, K], fp32, tag=tag, name=tag)

            iw = tmp("iw"); nc.vector.reciprocal(iw, widths)
            delta = tmp("delta"); TT(delta, heights, iw, OP.mult)
            rdelta = tmp("rdelta"); nc.vector.reciprocal(rdelta, delta)
            s = tmp("s")
            for t in range(T):
                TS(s[:, t, :], rdelta[:, t, :], delta[:, t, 0:1], None, OP.mult)
            sig = tmp("sig"); TT(sig, d0, d1, OP.add)
            STT(sig, delta, -2.0, sig, OP.mult, OP.add)
            sdelta = tmp("sdelta"); TT(sdelta, s, delta, OP.mult)
            ssig = tmp("ssig"); TT(ssig, s, sig, OP.mult)
            sh = tmp("sh"); TT(sh, s, heights, OP.mult)
            shd0 = tmp("shd0"); TT(shd0, sh, d0, OP.mult)
            t1 = tmp("t1"); TT(t1, y0, ssig, OP.mult)
            Nc1 = tmp("Nc1"); TT(Nc1, t1, shd0, OP.add)
            u1 = tmp("u1"); TT(u1, delta, d0, OP.subtract)
            u2 = tmp("u2"); TT(u2, sh, u1, OP.mult)
            Nc2 = tmp("Nc2"); TT(Nc2, u2, t1, OP.subtract)
            sd2 = tmp("sd2"); TT(sd2, sdelta, sdelta, OP.mult)
            Cc1 = tmp("Cc1"); STT(Cc1, sd2, 2.0, u1, OP.mult, OP.mult)
            Cc2 = tmp("Cc2"); TT(Cc2, sd2, sig, OP.mult)
            iw2 = tmp("iw2"); TT(iw2, iw, iw, OP.mult)

            # final coefs, ci-minor: coefcat[:, t, k, ci]
            # ci: 0=aN 1=aD 2=aC 3=bN 4=bD 5=bC  (sq triple, lin triple)
            coefcat = singles.tile([128, T, K, 6], fp32)
            aN = coefcat[:, :, :, 0]; TT(aN, Nc2, iw2, OP.mult)
            aD = coefcat[:, :, :, 1]; STT(aD, ssig, -1.0, iw2, OP.mult, OP.mult)
            aC = coefcat[:, :, :, 2]; TT(aC, Cc2, iw2, OP.mult)
            bN = coefcat[:, :, :, 3]; TT(bN, Nc1, iw, OP.mult)
            bD = coefcat[:, :, :, 4]; TT(bD, ssig, iw, OP.mult)
            bC = coefcat[:, :, :, 5]; TT(bC, Cc1, iw, OP.mult)

            # per-row constants (poly order N, D, C)
            constcat = singles.tile([128, 3, T], fp32)
            constN = constcat[:, 0, :]
            TT(constN, y0[:, :, 0], sdelta[:, :, 0], OP.mult)
            constD = constcat[:, 1, :]
            nc.vector.tensor_copy(constD, sdelta[:, :, 0])
            constC = constcat[:, 2, :]
            TT(constC, sd2[:, :, 0], d0[:, :, 0], OP.mult)

            # fold sum_k b_k * kx_k into the constants (streams are raw t_k)
            for cst, b in ((constN, bN), (constD, bD), (constC, bC)):
                bx = tmp("bx"); TT(bx, b, kx, OP.mult)
                sbx = singles.tile([128, T], fp32, tag="sbx", name="sbx")
                nc.vector.tensor_reduce(sbx, bx, axis=AX.X, op=OP.add)
                TT(cst, cst, sbx, OP.subtract)

            # ===== CPK2[p=(32*po+b), t, Q] = const_po[32Q+b, t] via PE ====
            psCPK = pssetup.tile([96, T * QQ], fp32, tag="ps", name="psCPK")
            for t in range(T):
                cE4 = work.tile([128, 3, 32], fp32, tag="cE4", name="cE4")
                cin = constcat[:, :, t].unsqueeze(2).broadcast_to([128, 3, 32])
                m32 = mask32.unsqueeze(1).broadcast_to([128, 3, 32])
                TT(cE4, cin, m32, OP.mult)
                nc.tensor.matmul(psCPK[:, t * QQ:(t + 1) * QQ],
                                 cE4.rearrange("p a b -> p (a b)"), gqsel,
                                 start=True, stop=True)
            CPK2 = singles.tile([96, T, QQ], fp32)
            nc.scalar.copy(CPK2.rearrange("p a b -> p (a b)"), psCPK)

            # ===== stage coefcat to DRAM; gather into lane layout =======
            dramCF = dscr.tile([128, T * K * 6], fp32, name="dramCF")
            dma(out=dramCF, in_=coefcat.rearrange("p a b c -> p (a b c)"))

            # coefP4[p=(32m+b), Q, t, h, (s,po)]
            coefP4 = singles.tile([128, QQ, T, HH, 6], fp32)
            for t in range(T):
                for Qb in range(QQ):
                    base = dramCF.offset + 32 * Qb * (T * K * 6) + t * (K * 6)
                    src = bass.AP(tensor=dramCF.tensor, offset=base,
                                  ap=[[4 * 6, QQ], [T * K * 6, 32], [1, HH * 6]])
                    (nc.scalar.dma_start if (Qb + t) % 2 else dma)(out=coefP4[:, Qb, t, :, :], in_=src)

        # ================= main loop ===================================
        # Output-side partition scramble: poly/finale partition p holds batch
        # row 32*(p%4) + p//4 of the t-block, so each un-interleave is ONE
        # full-width SBUF->SBUF DMA (src (b,Q,j) order == dst p=(b,Q) order).
        # The y load and out/logdet stores use the same scrambled row AP.
        #
        # Finales are SOFTWARE-PIPELINED one t-block late: each engine's
        # in-order SEQ would otherwise head-of-line block the next t's
        # stream-gen behind finale ops that wait on the PSUM drain.
        def emit_finale(t, c, polys, ysl):
            Np, Dp, Cp = polys
            xsl = fin.tile([128, 512], fp32, tag="xsl", name="xsl")
            nc.gpsimd.tensor_scalar(xsl, ysl, -BOUND, BOUND, OP.max, OP.min)
            # xsl -> ee (in place), inz on DVE (2x mode)
            nc.gpsimd.tensor_tensor(xsl, ysl, xsl, OP.subtract)
            inz = fin.tile([128, 512], fp32, tag="inz", name="inz")
            TS(inz, xsl, 0.0, None, OP.is_equal)
            rD = fin.tile([128, 512], fp32, tag="rD", name="rD")
            nc.vector.reciprocal(rD, Dp)
            # Cp -> max(C,eps) -> C*rD*rD -> ln -> *inz  == logdet
            nc.gpsimd.tensor_scalar(Cp, Cp, 1e-12, None, OP.max)
            TT(Cp, Cp, rD, OP.mult)
            TT(Cp, Cp, rD, OP.mult)
            ld0 = fin.tile([128, 512], fp32, tag="ld0", name="ld0")
            nc.scalar.activation(ld0, Cp, AF.Ln)
            nc.gpsimd.tensor_tensor(ld0, ld0, inz, OP.mult)
            # Np -> N*rD -> + (y - xc)  == out
            out0 = fin.tile([128, 512], fp32, tag="out0", name="out0")
            nc.gpsimd.tensor_tensor(out0, Np, rD, OP.mult)
            nc.gpsimd.tensor_tensor(out0, out0, xsl, OP.add)
            oview = io["out"][t * 128:(t + 1) * 128, c * 512:(c + 1) * 512]\
                .rearrange("(q b) j -> b q j", q=QQ)
            dma(out=oview, in_=out0.rearrange("(b q) j -> b q j", q=QQ))
            lview = io["logdet"][t * 128:(t + 1) * 128, c * 512:(c + 1) * 512]\
                .rearrange("(q b) j -> b q j", q=QQ)
            nc.scalar.dma_start(
                out=lview, in_=ld0.rearrange("(b q) j -> b q j", q=QQ))

        pending = []
        with tc.tile_pool(name="psum_acc", bufs=2, space="PSUM") as psum_acc:
            for t in range(T):
                # stationaries for this t: LHS4[p, Q, s, h, po, b]
                # (s=0 -> sq coefs a, s=1 -> lin coefs b; col = 32*po + b)
                LHS4 = lhsp.tile([128, QQ, 2, HH, 3, 32], fp32r, tag="lhs",
                                 name=f"lhs{t}")
                for Qb in range(QQ):
                    cslice = coefP4[:, Qb, t, :, :]\
                        .rearrange("p h (s o) -> p s h o", s=2)\
                        .unsqueeze(4).broadcast_to([128, 2, HH, 3, 32])
                    m32b = mask32.unsqueeze(1).unsqueeze(1).unsqueeze(1)\
                        .broadcast_to([128, 2, HH, 3, 32])
                    TT(LHS4[:, Qb], cslice, m32b, OP.mult)

                # both c-chunks' accumulators live together: matmuls issue
                # Q-major back-to-back to keep the PE p-state streak long
                ACCs = [psum_acc.tile([96, QQ * 512], fp32, tag="ACC",
                                      name=f"ACC{t}_{c}") for c in range(CH)]
                for Qb in range(QQ):
                    yrep = work.tile([128, N], fp32, tag="yrep", name="yrep")
                    src = io["y"][t * 128 + 32 * Qb:t * 128 + 32 * Qb + 32, :]
                    dma(out=yrep.rearrange("(m b) j -> m b j", b=32),
                        in_=src.unsqueeze(0).broadcast_to([4, 32, N]))
                    for h in range(HH):
                        tk = work.tile([128, N], fp32r, tag="tk", name="tk")
                        TS(tk, yrep, KX4[:, Qb, h, t:t + 1],
                           KX14[:, Qb, h, t:t + 1], OP.max, OP.min)
                        usq = work.tile([128, N], fp32r, tag="usq", name="usq")
                        nc.scalar.activation(usq, tk, AF.Square,
                                             bias=NEGKX4[:, Qb, h, t:t + 1])
                        ll = LHS4[:, Qb, 1, h].rearrange("p a b -> p (a b)")
                        lq = LHS4[:, Qb, 0, h].rearrange("p a b -> p (a b)")
                        for c in range(CH):
                            slot = ACCs[c][:, Qb * 512:(Qb + 1) * 512]
                            sl = slice(c * 512, (c + 1) * 512)
                            mmr(slot, ll, tk[:, sl], start=(h == 0), stop=False)
                            mmr(slot, lq, usq[:, sl], start=False,
                                stop=(h == HH - 1))

                # drain previous t's finales while this t's PSUM fills
                for item in pending:
                    emit_finale(*item)
                pending = []

                for c in range(CH):
                    ACC = ACCs[c]
                    # PSUM -> SBUF with per-row consts folded in
                    SACC = fin.tile([96, QQ, 512], fp32, tag="SACC", name="SACC")
                    for Qb in range(QQ):
                        if Qb % 2 == 0:
                            nc.scalar.activation(
                                SACC[:, Qb, :], ACC[:, Qb * 512:(Qb + 1) * 512],
                                AF.Identity, bias=CPK2[:, t, Qb:Qb + 1])
                        else:
                            TS(SACC[:, Qb, :], ACC[:, Qb * 512:(Qb + 1) * 512],
                               CPK2[:, t, Qb:Qb + 1], None, OP.add)

                    # un-interleave: one full-width SBUF->SBUF DMA per poly
                    polys = []
                    for pi in range(3):
                        dstt = work.tile([128, 512], fp32, tag=f"poly{pi}",
                                         name=f"poly{pi}")
                        dq = nc.scalar.dma_start if pi == 1 else dma
                        dq(out=dstt, in_=SACC[32 * pi:32 * pi + 32, :, :])
                        polys.append(dstt)
                    ysl = work.tile([128, 512], fp32, tag="ysl", name="ysl")
                    dma(out=ysl,
                        in_=io["y"][t * 128:(t + 1) * 128,
                                    c * 512:(c + 1) * 512]
                        .rearrange("(q b) j -> b q j", q=QQ))
                    pending.append((t, c, polys, ysl))

        for item in pending:
            emit_finale(*item)


def kernel(cond, y, W1, b1, W2, b2, W3, b3):
    _ensure_path()
    from concourse.bass_utils import run_bass_kernel_spmd

    if "nc" not in _CACHE:
        _CACHE["nc"] = _build_nc()
    nc = _CACHE["nc"]

    cond = np.ascontiguousarray(cond, np.float32)
    y = np.ascontiguousarray(y, np.float32)
    shared = dict(W1=np.ascontiguousarray(W1, np.float32),
                  b1=np.ascontiguousarray(b1, np.float32),
                  W2=np.ascontiguousarray(W2, np.float32),
                  b2=np.ascontiguousarray(b2, np.float32),
                  W3=np.ascontiguousarray(W3, np.float32),
                  b3=np.ascontiguousarray(b3, np.float32))
    in_maps = []
    for i in range(NCORES):
        sl = slice(i * BL, (i + 1) * BL)
        in_maps.append(dict(cond=cond[sl], y=y[sl], **shared))
    res = run_bass_kernel_spmd(nc, in_maps, core_ids=list(range(NCORES)))
    out = np.concatenate([r["out"] for r in res.results], axis=0)
    ld = np.concatenate([r["logdet"] for r in res.results], axis=0)
    return out, ld
